# revision 6
# baseline (speedup 1.0000x reference)
"""Trainium2 Bass kernel for nn_AutoregressivePredictor.

Strategy
--------
Data-parallel over the 4096 independent timesteps: 8 cores x 512 timesteps.
Each timestep runs an 8-step autoregressive chain through one decoder layer.

Key algorithmic points:
  * KV caching: buffer row j never changes after it is written, so each step
    only computes q/k/v for the NEW row and attends over cached K/V.
  * Token tables: for steps p>=1 the new row is E[tok], so
    q/k/v = rope_p((rmsnorm(E)*n1 @ W)[tok]).  We precompute, once, on device:
    vE = rmsnorm(E)n1@Wv and rope-baked tables qE_p / kE_p for p=1..7.
    Per step this replaces three [512,1024]x[1024,1024] matmuls + rope with
    four row-gathers.
  * All matmuls run in fp32 (4 cycles/row): fp32r/bf16/fp16 were measured on
    HW to be far too imprecise for the ~2e-6 top-2 logit margins.
  * Activations flow feature-major [feat, batch]: every matmul uses the
    weight as stationary lhsT, so no transposes in the main chain.  Only ctx
    and cur (which arrive batch-major from attention/gathers) are transposed
    via the PE.
  * rsqrt for rmsnorm gets one Newton refinement (ACT Sqrt table is ~7e-6).

Self-contained: hardcodes all shapes; creates its own Bass program.
"""

import numpy as np

P = 128
H = 1024
NH = 16
HD = 64
I = 4096
V = 1024
T_FULL = 4096
NSTEPS = 8
N_CORES = 8
EPS = 1e-6
THETA = 10000.0
HKT = H // P          # 8 k-tiles over hidden dim
IKT = I // P          # 32 k-tiles over intermediate dim
IH = IKT // 2         # 16 i-tiles per half


def _rope_tables():
    """Position-expanded cos/sin tables [NSTEPS, H], fp32, matching reference.

    For head-dim layout [h*64+d]:
      out[d] = x[d]*cos[d] + rot(x)[d]*sin[d]
      rot(x)[d] = -x[d+32] (d<32), x[d-32] (d>=32)
    We implement rot(x)*sin as swap(x)*sin' with
      sin'[d] = -sin_half[d] (d<32), +sin_half[d-32] (d>=32)
    cos[d] = cos_half[d % 32].
    The q-side tables additionally fold in the 1/sqrt(HD) score scale.
    """
    inv_freq = (1.0 / (THETA ** (np.arange(0, HD, 2, dtype=np.float32) / np.float32(HD)))).astype(np.float32)
    cos_t = np.zeros((NSTEPS, H), np.float32)
    sin_t = np.zeros((NSTEPS, H), np.float32)
    for p in range(NSTEPS):
        ang = (np.float32(p) * inv_freq).astype(np.float32)  # [32]
        ch = np.cos(ang).astype(np.float32)
        sh = np.sin(ang).astype(np.float32)
        cos64 = np.concatenate([ch, ch])                      # [64]
        sinp64 = np.concatenate([-sh, sh])                    # sign-folded
        cos_t[p] = np.tile(cos64, NH)
        sin_t[p] = np.tile(sinp64, NH)
    scale = np.float32(1.0 / np.sqrt(np.float32(HD)))
    return cos_t * scale, sin_t * scale, cos_t, sin_t


def build_kernel(nc, tc, bass_mod, mybir, TL):
    """Emit the full per-core program. TL = local timesteps (multiple of 128)."""
    from contextlib import ExitStack
    from concourse.masks import make_identity

    BT = TL // P
    dt = mybir.dt
    AF = mybir.ActivationFunctionType
    OP = mybir.AluOpType
    AX = mybir.AxisListType

    # ---------------- I/O ----------------
    def din(name, shape):
        return nc.dram_tensor(name, shape, dt.float32, kind="ExternalInput").ap()

    x0_t_d = din("x0_t", [H, TL])
    Wq_d = din("Wq", [H, H]); Wk_d = din("Wk", [H, H]); Wv_d = din("Wv", [H, H])
    Wo_d = din("Wo", [H, H])
    Wg_d = din("Wg", [H, I]); Wu_d = din("Wu", [H, I]); Wd_d = din("Wd", [I, H])
    E_d = din("E", [V, H]); Wout_d = din("Wout", [H, V])
    n1_d = din("n1", [H]); n2_d = din("n2", [H]); bout_d = din("bout", [V])
    cq_d = din("rope_cos_q", [NSTEPS, H]); sq_d = din("rope_sin_q", [NSTEPS, H])
    ck_d = din("rope_cos_k", [NSTEPS, H]); sk_d = din("rope_sin_k", [NSTEPS, H])
    toks_d = nc.dram_tensor("toks", [TL, NSTEPS], dt.int32, kind="ExternalOutput").ap()

    # ---------------- scratch DRAM ----------------
    def dscratch(name, shape):
        return nc.dram_tensor(name, shape, dt.float32, kind="Internal").ap()

    qE_raw_d = dscratch("qE_raw", [V, H])
    kE_raw_d = dscratch("kE_raw", [V, H])
    vE_d = dscratch("vE_tab", [V, H])
    qEp_d = [None] + [dscratch(f"qE_p{p}", [V, H]) for p in range(1, NSTEPS)]
    kEp_d = [None] + [dscratch(f"kE_p{p}", [V, H]) for p in range(1, NSTEPS)]
    Kc_d = dscratch("K_cache", [TL, NSTEPS, H])
    Vc_d = dscratch("V_cache", [TL, NSTEPS, H])
    cur_f_d = dscratch("cur_f", [H, TL])
    h_f_d = dscratch("h_f", [H, TL])

    ctx = ExitStack()
    with ctx:
        # -------- pools --------
        sb = ctx.enter_context(tc.tile_pool(name="sb", bufs=1))
        ps_pool = ctx.enter_context(tc.tile_pool(name="ps", bufs=1, space="PSUM"))

        def til(shape, tag, bufs, dtype=dt.float32, name=None):
            return sb.tile(shape, dtype, tag=tag, bufs=bufs, name=name or tag)

        def psum(tag="ps", bufs=4, shape=(P, 512), name=None):
            return ps_pool.tile(list(shape), dt.float32, space="PSUM", tag=tag,
                                bufs=bufs, name=name or tag)

        # -------- constants --------
        ident = til([P, P], "consts_id", 1, name="ident")
        make_identity(nc, ident[:])
        ones_m = til([1, P], "consts_ones", 1, name="ones_m")
        nc.vector.memset(ones_m[:], 1.0)
        invH = til([P, 1], "consts_invH", 1, name="invH")
        nc.vector.memset(invH[:], 1.0 / H)
        n1_sb = til([P, HKT], "consts_n1", 1, name="n1_sb")
        nc.sync.dma_start(n1_sb[:], n1_d.rearrange("(kt p) -> p kt", p=P))
        n2_sb = til([P, HKT], "consts_n2", 1, name="n2_sb")
        nc.sync.dma_start(n2_sb[:], n2_d.rearrange("(kt p) -> p kt", p=P))

        def bcast_row(row_sb, out_tile, width):
            """Broadcast [1, width] SBUF row across 128 partitions via PE."""
            bps = psum(tag="bc", bufs=1, shape=(P, 1024), name="bps")
            for nch in range(0, width, 512):
                w = min(512, width - nch)
                nc.tensor.matmul(bps[:, nch:nch + w], lhsT=ones_m[:1, :],
                                 rhs=row_sb[:1, nch:nch + w], start=True, stop=True)
            nc.scalar.copy(out_tile[:, :width], bps[:, :width])

        n1row = til([1, H], "rowbuf", 2, name="n1row")
        nc.sync.dma_start(n1row[:1, :], n1_d[None, :])
        n1b = til([P, H], "consts_n1b", 1, name="n1b")
        bcast_row(n1row, n1b, H)
        boutrow = til([1, V], "rowbuf", 2, name="boutrow")
        nc.sync.dma_start(boutrow[:1, :], bout_d[None, :])
        boutb = til([P, V], "consts_boutb", 1, name="boutb")
        bcast_row(boutrow, boutb, V)

        # ============ helpers ============
        def newton_rsqrt(var_ap, r_ap, tmp_alloc):
            """r = rsqrt(var), refined.  var_ap/r_ap [Q,1] or [1,Q]; tmp_alloc()
            returns fresh tiles of the same shape."""
            s = tmp_alloc()
            nc.scalar.activation(s, var_ap, AF.Sqrt)
            y0 = tmp_alloc()
            nc.vector.reciprocal(y0, s)
            t = tmp_alloc()
            nc.vector.tensor_mul(t, y0, y0)
            nc.vector.tensor_mul(t, t, var_ap)
            nc.vector.tensor_scalar(t, t, -0.5, 1.5, OP.mult, OP.add)
            nc.vector.tensor_mul(r_ap, y0, t)

        def rms_from_dram(src_d, n_sb_tile, out_act):
            """out[kt] = src[kt] * n[kt] * rsqrt(mean_feat(src^2)+eps).
            src_d: f-major [H, TL] DRAM; out_act: SBUF [128, HKT, TL]."""
            psv = psum(tag="psv", bufs=1, shape=(1, 512), name="psv")
            for kt in range(HKT):
                hkt = til([P, TL], "tdr", 3, name="hkt")
                nc.sync.dma_start(hkt[:], src_d[kt * P:(kt + 1) * P, :])
                sqk = til([P, TL], "tdr", 3, name="sqk")
                nc.scalar.activation(sqk[:], hkt[:], AF.Square)
                nc.tensor.matmul(psv[:1, :TL], lhsT=invH[:, :1], rhs=sqk[:],
                                 start=(kt == 0), stop=(kt == HKT - 1),
                                 skip_group_check=True)
            v_sb = til([1, TL], "sr", 3, name="v_sb")
            nc.vector.tensor_scalar(v_sb[:1, :], psv[:1, :TL], EPS, None, OP.add)
            r = til([1, TL], "sr", 3, name="r_sb")
            newton_rsqrt(v_sb[:1, :], r[:1, :],
                         lambda: til([1, TL], "sr", 3, name="nt")[:1, :])
            rb = psum(tag="ps", bufs=4, name="rbps")
            nc.tensor.matmul(rb[:, :TL], lhsT=ones_m[:1, :], rhs=r[:1, :TL],
                             start=True, stop=True)
            for kt in range(HKT):
                hkt = til([P, TL], "tdr", 3, name="hkt2")
                nc.sync.dma_start(hkt[:], src_d[kt * P:(kt + 1) * P, :])
                nc.vector.scalar_tensor_tensor(out_act[:, kt, :], hkt[:],
                                               n_sb_tile[:, kt:kt + 1], rb[:, :TL],
                                               op0=OP.mult, op1=OP.mult)

        def mm_weights(W_dram, rhs_act, drain_fn, n_mt=HKT, n_kt=HKT):
            """For each output m-tile: psum = sum_kt W[:,mt].T @ rhs[kt]; drain_fn(mt, ps)."""
            for mt in range(n_mt):
                wt = til([P, n_kt, P], "w", 2, name="wt")
                nc.sync.dma_start(wt[:], W_dram[:, mt * P:(mt + 1) * P]
                                  .rearrange("(kt p) m -> p kt m", p=P))
                ps = psum(name="mmps")
                for kt in range(n_kt):
                    nc.tensor.matmul(ps[:, :TL], lhsT=wt[:, kt, :], rhs=rhs_act[:, kt, :],
                                     start=(kt == 0), stop=(kt == n_kt - 1))
                drain_fn(mt, ps)

        def transpose_b2f_single(src_b, bt, dst_act):
            """[128, H] batch-major tile -> dst_act[:, kt, bt*128:(bt+1)*128] f-major."""
            for g in range(2):
                ps = psum(name="tps")
                for i4 in range(4):
                    kt = g * 4 + i4
                    nc.tensor.transpose(ps[:, i4 * P:(i4 + 1) * P],
                                        src_b[:, kt * P:(kt + 1) * P], ident[:])
                for i4 in range(4):
                    kt = g * 4 + i4
                    eng = nc.scalar if (kt % 2 == 0) else nc.vector
                    if eng is nc.scalar:
                        nc.scalar.copy(dst_act[:, kt, bt * P:(bt + 1) * P],
                                       ps[:, i4 * P:(i4 + 1) * P])
                    else:
                        nc.vector.tensor_copy(dst_act[:, kt, bt * P:(bt + 1) * P],
                                              ps[:, i4 * P:(i4 + 1) * P])

        def transpose_b2f_to_dram(src_b, bt, dst_d):
            """[128, H] batch-major tile -> dst_d[:, bt*128:(bt+1)*128] f-major DRAM."""
            for g in range(2):
                ps = psum(name="tps2")
                for i4 in range(4):
                    kt = g * 4 + i4
                    nc.tensor.transpose(ps[:, i4 * P:(i4 + 1) * P],
                                        src_b[:, kt * P:(kt + 1) * P], ident[:])
                cp = til([P, 512], "tdr", 3, name="tcp")
                nc.scalar.copy(cp[:], ps[:])
                for i4 in range(4):
                    kt = g * 4 + i4
                    nc.sync.dma_start(dst_d[kt * P:(kt + 1) * P, bt * P:(bt + 1) * P],
                                      cp[:, i4 * P:(i4 + 1) * P])

        def transpose_f2b(src_act, bt, out_b):
            """f-major [128,HKT,TL] column block bt -> [128, H] batch-major tile."""
            for g in range(2):
                ps = psum(name="tps3")
                for i4 in range(4):
                    kt = g * 4 + i4
                    nc.tensor.transpose(ps[:, i4 * P:(i4 + 1) * P],
                                        src_act[:, kt, bt * P:(bt + 1) * P], ident[:])
                nc.scalar.copy(out_b[:, g * 512:(g + 1) * 512], ps[:])

        def gather_rows(table_d, tok_ap, out_tile):
            nc.gpsimd.indirect_dma_start(
                out=out_tile[:], out_offset=None, in_=table_d[:, :],
                in_offset=bass_mod.IndirectOffsetOnAxis(ap=tok_ap, axis=0))

        def rope_apply(src, cosb, sinb, out):
            """out = src*cos + swap(src)*sin' ; all [128, H]."""
            nc.vector.tensor_mul(out[:], src[:], cosb[:])
            t2 = til([P, H], "tmp", 3, name="ropet2")
            src3 = src[:].rearrange("p (h d) -> p h d", d=HD)
            sin3 = sinb[:].rearrange("p (h d) -> p h d", d=HD)
            t23 = t2[:].rearrange("p (h d) -> p h d", d=HD)
            nc.gpsimd.tensor_tensor(t23[:, :, 0:HD // 2], src3[:, :, HD // 2:HD],
                                    sin3[:, :, 0:HD // 2], op=OP.mult)
            nc.gpsimd.tensor_tensor(t23[:, :, HD // 2:HD], src3[:, :, 0:HD // 2],
                                    sin3[:, :, HD // 2:HD], op=OP.mult)
            nc.vector.tensor_add(out[:], out[:], t2[:])

        def load_rope_bcast(p):
            out = {}
            for nm, tab in (("cq", cq_d), ("sq", sq_d), ("ck", ck_d), ("sk", sk_d)):
                row = til([1, H], "rowbuf", 2, name=f"row_{nm}")
                nc.sync.dma_start(row[:1, :], tab[p:p + 1, :])
                bt_tile = til([P, H], "g4", 6, name=f"rb_{nm}")
                bcast_row(row, bt_tile, H)
                out[nm] = bt_tile
            return out

        # ============ Phase A: token tables ============
        # A1: rmsnorm rows of E (v-major) -> transpose into rmsEt [h, v] f-major
        rmsEt = til([P, HKT, H], "big32", 1, name="rmsEt")  # [128, ht, v] 32KB
        for vt in range(HKT):
            ev = til([P, H], "g4", 6, name="ev")
            nc.sync.dma_start(ev[:], E_d[vt * P:(vt + 1) * P, :])
            sqe = til([P, H], "tmp", 3, name="sqe")
            nc.scalar.activation(sqe[:], ev[:], AF.Square)
            ssum = til([P, 1], "s1", 3, name="ssum")
            nc.vector.tensor_reduce(ssum[:], sqe[:], axis=AX.X, op=OP.add)
            var = til([P, 1], "s1", 3, name="var")
            nc.vector.tensor_scalar(var[:], ssum[:], 1.0 / H, EPS, OP.mult, OP.add)
            r1 = til([P, 1], "s1", 3, name="r1")
            newton_rsqrt(var[:], r1[:], lambda: til([P, 1], "s1", 3, name="nt1")[:])
            rme = til([P, H], "g4", 6, name="rme")
            nc.vector.scalar_tensor_tensor(rme[:], ev[:], r1[:, :1], n1b[:],
                                           op0=OP.mult, op1=OP.mult)
            for g in range(2):
                ps = psum(name="taps")
                for i4 in range(4):
                    ht = g * 4 + i4
                    nc.tensor.transpose(ps[:, i4 * P:(i4 + 1) * P],
                                        rme[:, ht * P:(ht + 1) * P], ident[:])
                for i4 in range(4):
                    ht = g * 4 + i4
                    nc.scalar.copy(rmsEt[:, ht, vt * P:(vt + 1) * P],
                                   ps[:, i4 * P:(i4 + 1) * P])
        # A2: raw tables = rmsEt.T @ W  ->  DRAM (v-major rows)
        for W_d, dst in ((Wq_d, qE_raw_d), (Wk_d, kE_raw_d), (Wv_d, vE_d)):
            for nch in range(2):
                wt = til([P, HKT, 512], "w", 2, name="twt")
                nc.sync.dma_start(wt[:], W_d[:, nch * 512:(nch + 1) * 512]
                                  .rearrange("(kt p) n -> p kt n", p=P))
                for mt in range(HKT):
                    ps = psum(name="tmmps")
                    for kt in range(HKT):
                        nc.tensor.matmul(ps[:], lhsT=rmsEt[:, kt, mt * P:(mt + 1) * P],
                                         rhs=wt[:, kt, :], start=(kt == 0), stop=(kt == HKT - 1))
                    ot = til([P, 512], "tdr", 3, name="tot")
                    nc.scalar.copy(ot[:], ps[:])
                    nc.sync.dma_start(dst[mt * P:(mt + 1) * P, nch * 512:(nch + 1) * 512], ot[:])
        # A3: rope-baked tables for p=1..7
        for p in range(1, NSTEPS):
            rbc = load_rope_bcast(p)
            for raw_d, dst_d, cb, sbn in ((qE_raw_d, qEp_d[p], "cq", "sq"),
                                          (kE_raw_d, kEp_d[p], "ck", "sk")):
                for vt in range(HKT):
                    raw = til([P, H], "g4", 6, name="rawv")
                    nc.sync.dma_start(raw[:], raw_d[vt * P:(vt + 1) * P, :])
                    outp = til([P, H], "g4", 6, name="outp")
                    rope_apply(raw, rbc[cb], rbc[sbn], outp)
                    nc.sync.dma_start(dst_d[vt * P:(vt + 1) * P, :], outp[:])

        # ============ per-step tail: rms2 + MLP + logits + argmax ============
        def mlp_and_logits(p, tok_tiles):
            """h is in h_f_d (DRAM, f-major).  Produces tok_tiles + toks output."""
            x2 = til([P, HKT, TL], "act16", 3, name="x2")
            rms_from_dram(h_f_d, n2_sb, x2)
            hf = til([P, HKT, TL], "act16", 3, name="hf")
            for ihalf in range(2):
                silu = til([P, IH, TL], "big32", 1, name="silu")
                for im in range(IH):
                    imt = ihalf * IH + im
                    wg = til([P, HKT, P], "w", 2, name="wg")
                    nc.sync.dma_start(wg[:], Wg_d[:, imt * P:(imt + 1) * P]
                                      .rearrange("(kt p) m -> p kt m", p=P))
                    wu = til([P, HKT, P], "w", 2, name="wu")
                    nc.sync.dma_start(wu[:], Wu_d[:, imt * P:(imt + 1) * P]
                                      .rearrange("(kt p) m -> p kt m", p=P))
                    psg = psum(name="psg")
                    for kt in range(HKT):
                        nc.tensor.matmul(psg[:, :TL], lhsT=wg[:, kt, :], rhs=x2[:, kt, :],
                                         start=(kt == 0), stop=(kt == HKT - 1))
                    psu = psum(name="psu")
                    for kt in range(HKT):
                        nc.tensor.matmul(psu[:, :TL], lhsT=wu[:, kt, :], rhs=x2[:, kt, :],
                                         start=(kt == 0), stop=(kt == HKT - 1))
                    sg = til([P, TL], "tdr", 3, name="sg")
                    nc.scalar.activation(sg[:, :TL], psg[:, :TL], AF.Sigmoid)
                    nc.vector.tensor_mul(sg[:, :TL], sg[:, :TL], psg[:, :TL])
                    nc.vector.tensor_mul(silu[:, im, :], sg[:, :TL], psu[:, :TL])
                for hmt in range(HKT):
                    wd = til([P, IH, P], "w", 2, name="wd")
                    nc.sync.dma_start(wd[:], Wd_d[ihalf * (I // 2):(ihalf + 1) * (I // 2),
                                                  hmt * P:(hmt + 1) * P]
                                      .rearrange("(kt p) m -> p kt m", p=P))
                    ps = psum(name="psd")
                    for kt in range(IH):
                        nc.tensor.matmul(ps[:, :TL], lhsT=wd[:, kt, :], rhs=silu[:, kt, :],
                                         start=(kt == 0), stop=(kt == IH - 1))
                    if ihalf == 0:
                        hres = til([P, TL], "tdr", 3, name="hres")
                        nc.sync.dma_start(hres[:], h_f_d[hmt * P:(hmt + 1) * P, :])
                        nc.vector.tensor_add(hf[:, hmt, :], ps[:, :TL], hres[:])
                    else:
                        nc.vector.tensor_add(hf[:, hmt, :], ps[:, :TL], hf[:, hmt, :])
            # logits (batch-major) + argmax; keep both Wout chunks resident
            wout = []
            for nch in range(2):
                wt = til([P, HKT, 512], "w", 2, name=f"wout{nch}")
                nc.sync.dma_start(wt[:], Wout_d[:, nch * 512:(nch + 1) * 512]
                                  .rearrange("(kt p) n -> p kt n", p=P))
                wout.append(wt)
            for bt in range(BT):
                lg = til([P, V], "lg", 2, name="lg")
                for nch in range(2):
                    ps = psum(name="pslg")
                    for kt in range(HKT):
                        nc.tensor.matmul(ps[:], lhsT=hf[:, kt, bt * P:(bt + 1) * P],
                                         rhs=wout[nch][:, kt, :],
                                         start=(kt == 0), stop=(kt == HKT - 1))
                    nc.vector.tensor_add(lg[:, nch * 512:(nch + 1) * 512], ps[:],
                                         boutb[:, nch * 512:(nch + 1) * 512])
                mx = til([P, 8], "mx", 2, name="mx")
                nc.vector.max(mx[:], lg[:])
                idx = til([P, 8], "idx", 2, dtype=dt.uint32, name="idx")
                nc.vector.max_index(idx[:], mx[:], lg[:])
                tok = til([P, 1], "tok", 2 * BT + 1, dtype=dt.uint32, name="tok")
                nc.vector.tensor_copy(tok[:], idx[:, 0:1])
                tok_tiles[bt] = tok
                nc.sync.dma_start(toks_d[bt * P:(bt + 1) * P, p:p + 1],
                                  tok[:].bitcast(dt.int32))

        # ============ Phase B: step 0 ============
        x1 = til([P, HKT, TL], "act16", 3, name="x1")
        rms_from_dram(x0_t_d, n1_sb, x1)
        k0 = til([P, HKT, TL], "act16", 3, name="k0")
        mm_weights(Wk_d, x1, lambda mt, ps: nc.scalar.copy(k0[:, mt, :], ps[:, :TL]))
        v0 = til([P, HKT, TL], "act16", 3, name="v0")
        mm_weights(Wv_d, x1, lambda mt, ps: nc.scalar.copy(v0[:, mt, :], ps[:, :TL]))
        # KV cache write (position 0; rope at pos 0 is identity)
        for bt in range(BT):
            kb = til([P, H], "kv", 3, name="kb")
            transpose_f2b(k0, bt, kb)
            nc.sync.dma_start(Kc_d[bt * P:(bt + 1) * P, 0, :], kb[:])
            vb = til([P, H], "kv", 3, name="vb")
            transpose_f2b(v0, bt, vb)
            nc.sync.dma_start(Vc_d[bt * P:(bt + 1) * P, 0, :], vb[:])

        # h = x0 + v0 @ Wo  (attention at step 0 is identity: ctx = v0)
        def drain_h_step0(mt, ps):
            x0kt = til([P, TL], "tdr", 3, name="x0kt")
            nc.sync.dma_start(x0kt[:], x0_t_d[mt * P:(mt + 1) * P, :])
            hsb = til([P, TL], "tdr", 3, name="hsb")
            nc.vector.tensor_add(hsb[:], ps[:, :TL], x0kt[:])
            nc.sync.dma_start(h_f_d[mt * P:(mt + 1) * P, :], hsb[:])
        mm_weights(Wo_d, v0, drain_h_step0)
        tok_tiles = [None] * BT
        mlp_and_logits(0, tok_tiles)

        # ============ Phase C: steps 1..7 ============
        for p in range(1, NSTEPS):
            ctx_t = til([P, HKT, TL], "act16", 3, name="ctx_t")
            for bt in range(BT):
                tokap = tok_tiles[bt][:, :1]
                # gathers
                cb = til([P, H], "g4", 6, name="cb")
                gather_rows(E_d, tokap, cb)
                transpose_b2f_to_dram(cb, bt, cur_f_d)
                qp = til([P, H], "g4", 6, name="qp")
                gather_rows(qEp_d[p], tokap, qp)
                kp = til([P, H], "g4", 6, name="kp")
                gather_rows(kEp_d[p], tokap, kp)
                vp = til([P, H], "g4", 6, name="vp")
                gather_rows(vE_d, tokap, vp)
                nc.sync.dma_start(Kc_d[bt * P:(bt + 1) * P, p, :], kp[:])
                nc.sync.dma_start(Vc_d[bt * P:(bt + 1) * P, p, :], vp[:])
                # ---- attention ----
                s_sb = til([P, P], "s", 2, name="s_sb")
                for j in range(p + 1):
                    if j < p:
                        Kj = til([P, H], "kv", 3, name="Kj")
                        nc.sync.dma_start(Kj[:], Kc_d[bt * P:(bt + 1) * P, j, :])
                    else:
                        Kj = kp
                    tmp = til([P, H], "tmp", 3, name="atmp")
                    nc.vector.tensor_mul(tmp[:], Kj[:], qp[:])
                    nc.vector.tensor_reduce(
                        s_sb[:, j * NH:(j + 1) * NH],
                        tmp[:].rearrange("p (h d) -> p h d", d=HD),
                        axis=AX.X, op=OP.add)
                m16 = til([P, NH], "m16", 4, name="m16")
                nc.vector.tensor_reduce(
                    m16[:], s_sb[:, :(p + 1) * NH].rearrange("p (j h) -> p h j", h=NH),
                    axis=AX.X, op=OP.max)
                e_sb = til([P, P], "s", 2, name="e_sb")
                nc.vector.tensor_tensor(
                    e_sb[:, :(p + 1) * NH], s_sb[:, :(p + 1) * NH],
                    m16[:].unsqueeze(1).to_broadcast([P, p + 1, NH]),
                    op=OP.subtract)
                nc.scalar.activation(e_sb[:, :(p + 1) * NH], e_sb[:, :(p + 1) * NH], AF.Exp)
                z16 = til([P, NH], "m16", 4, name="z16")
                nc.vector.tensor_reduce(
                    z16[:], e_sb[:, :(p + 1) * NH].rearrange("p (j h) -> p h j", h=NH),
                    axis=AX.X, op=OP.add)
                rz = til([P, NH], "m16", 4, name="rz")
                nc.vector.reciprocal(rz[:], z16[:])
                nc.vector.tensor_tensor(
                    e_sb[:, :(p + 1) * NH], e_sb[:, :(p + 1) * NH],
                    rz[:].unsqueeze(1).to_broadcast([P, p + 1, NH]),
                    op=OP.mult)
                cx = til([P, H], "cx", 2, name="cx")
                for j in range(p + 1):
                    if j < p:
                        Vj = til([P, H], "kv", 3, name="Vj")
                        nc.sync.dma_start(Vj[:], Vc_d[bt * P:(bt + 1) * P, j, :])
                    else:
                        Vj = vp
                    aj = e_sb[:, j * NH:(j + 1) * NH].unsqueeze(2).to_broadcast([P, NH, HD])
                    if j == 0:
                        nc.vector.tensor_tensor(
                            cx[:].rearrange("p (h d) -> p h d", d=HD),
                            Vj[:].rearrange("p (h d) -> p h d", d=HD), aj, op=OP.mult)
                    else:
                        tmp = til([P, H], "tmp", 3, name="atmp2")
                        nc.vector.tensor_tensor(
                            tmp[:].rearrange("p (h d) -> p h d", d=HD),
                            Vj[:].rearrange("p (h d) -> p h d", d=HD), aj, op=OP.mult)
                        nc.vector.tensor_add(cx[:], cx[:], tmp[:])
                transpose_b2f_single(cx, bt, ctx_t)

            # h = cur + ctx @ Wo
            def drain_h(mt, ps):
                ckt = til([P, TL], "tdr", 3, name="ckt")
                nc.sync.dma_start(ckt[:], cur_f_d[mt * P:(mt + 1) * P, :])
                hsb = til([P, TL], "tdr", 3, name="hsb2")
                nc.vector.tensor_add(hsb[:], ps[:, :TL], ckt[:])
                nc.sync.dma_start(h_f_d[mt * P:(mt + 1) * P, :], hsb[:])
            mm_weights(Wo_d, ctx_t, drain_h)
            mlp_and_logits(p, tok_tiles)

    return nc


_CACHED = {}


def _build(TL):
    if TL in _CACHED:
        return _CACHED[TL]
    import concourse.bass as bass
    import concourse.tile as tile
    from concourse import bacc, mybir
    nc = bacc.Bacc("TRN2", target_bir_lowering=False, debug=False, num_devices=N_CORES)
    with tile.TileContext(nc) as tc:
        build_kernel(nc, tc, bass, mybir, TL)
    nc.compile()
    _CACHED[TL] = nc
    return nc


def make_in_maps(inputs, n_cores=N_CORES, TL=None):
    """Shard/augment the full inputs into per-core in_maps."""
    x0 = np.ascontiguousarray(np.asarray(inputs["chunk_hidden_states"], dtype=np.float32)[0])  # [T, H]
    T = x0.shape[0]
    if TL is None:
        TL = T // n_cores
    cq, sq, ck, sk = _rope_tables()
    shared = {
        "Wq": np.asarray(inputs["Wq"], np.float32), "Wk": np.asarray(inputs["Wk"], np.float32),
        "Wv": np.asarray(inputs["Wv"], np.float32), "Wo": np.asarray(inputs["Wo"], np.float32),
        "Wg": np.asarray(inputs["Wg"], np.float32), "Wu": np.asarray(inputs["Wu"], np.float32),
        "Wd": np.asarray(inputs["Wd"], np.float32), "E": np.asarray(inputs["E"], np.float32),
        "Wout": np.asarray(inputs["Wout"], np.float32),
        "n1": np.asarray(inputs["n1"], np.float32), "n2": np.asarray(inputs["n2"], np.float32),
        "bout": np.asarray(inputs["bout"], np.float32),
        "rope_cos_q": cq, "rope_sin_q": sq, "rope_cos_k": ck, "rope_sin_k": sk,
    }
    in_maps = []
    for c in range(n_cores):
        m = dict(shared)
        m["x0_t"] = np.ascontiguousarray(x0[c * TL:(c + 1) * TL, :].T)
        in_maps.append(m)
    return in_maps, TL


def kernel(**inputs):
    from concourse.bass_utils import run_bass_kernel_spmd
    in_maps, TL = make_in_maps(inputs)
    nc = _build(TL)
    res = run_bass_kernel_spmd(nc, in_maps, core_ids=list(range(N_CORES)))
    toks = np.concatenate([r["toks"] for r in res.results], axis=0)  # [T, 8]
    return toks.astype(np.int32)


# revision 13
# speedup vs baseline: 1.0207x; 1.0207x over previous
"""Trainium2 Bass kernel for nn_AutoregressivePredictor.

Strategy
--------
Data-parallel over the 4096 independent timesteps: 8 cores x 512 timesteps.
Each timestep runs an 8-step autoregressive chain through one decoder layer.

Key algorithmic points:
  * KV caching: buffer row j never changes after it is written, so each step
    only computes q/k/v for the NEW row and attends over cached K/V.
  * Token tables: for steps p>=1 the new row is E[tok], so
    q/k/v = rope_p((rmsnorm(E)*n1 @ W)[tok]).  We precompute, once, on device:
    vE = rmsnorm(E)n1@Wv and rope-baked tables qE_p / kE_p for p=1..7.
    Per step this replaces three [512,1024]x[1024,1024] matmuls + rope with
    four row-gathers.
  * fp16 hi/lo split matmuls (3 passes: hi@hi + 2^-11*(hi@lo + lo@hi)):
    measured max error 5.3e-7 (same as the PE's fp32 mode) at 3 cycles/row
    instead of fp32's 4.  Raw fp32r/bf16/fp16 were measured far too imprecise
    for the ~2e-6 top-2 logit margins.  Weights are split host-side; the
    activation operand is split on device (3 cheap elementwise passes).
  * Activations flow feature-major [feat, batch]: every matmul uses the
    weight as stationary lhsT, so no transposes in the main chain.  Only ctx
    and cur (which arrive batch-major from attention/gathers) are transposed
    via the PE.
  * rsqrt for rmsnorm gets one Newton refinement (ACT Sqrt table is ~7e-6).

Self-contained: hardcodes all shapes; creates its own Bass program.
"""

import numpy as np

P = 128
H = 1024
NH = 16
HD = 64
I = 4096
V = 1024
T_FULL = 4096
NSTEPS = 8
N_CORES = 8
EPS = 1e-6
THETA = 10000.0
HKT = H // P          # 8 k-tiles over hidden dim
IKT = I // P          # 32 k-tiles over intermediate dim
IH = IKT // 2         # 16 i-tiles per half
S = 2048.0            # lo-part scale (2^11) keeps fp16 lo parts normal


def _rope_tables():
    """Position-expanded cos/sin tables [NSTEPS, H], fp32, matching reference.

    For head-dim layout [h*64+d]:
      out[d] = x[d]*cos[d] + rot(x)[d]*sin[d]
      rot(x)[d] = -x[d+32] (d<32), x[d-32] (d>=32)
    We implement rot(x)*sin as swap(x)*sin' with
      sin'[d] = -sin_half[d] (d<32), +sin_half[d-32] (d>=32)
    cos[d] = cos_half[d % 32].
    The q-side tables additionally fold in the 1/sqrt(HD) score scale.
    """
    inv_freq = (1.0 / (THETA ** (np.arange(0, HD, 2, dtype=np.float32) / np.float32(HD)))).astype(np.float32)
    cos_t = np.zeros((NSTEPS, H), np.float32)
    sin_t = np.zeros((NSTEPS, H), np.float32)
    for p in range(NSTEPS):
        ang = (np.float32(p) * inv_freq).astype(np.float32)  # [32]
        ch = np.cos(ang).astype(np.float32)
        sh = np.sin(ang).astype(np.float32)
        cos64 = np.concatenate([ch, ch])                      # [64]
        sinp64 = np.concatenate([-sh, sh])                    # sign-folded
        cos_t[p] = np.tile(cos64, NH)
        sin_t[p] = np.tile(sinp64, NH)
    scale = np.float32(1.0 / np.sqrt(np.float32(HD)))
    return cos_t * scale, sin_t * scale, cos_t, sin_t


WEIGHTS = ["Wq", "Wk", "Wv", "Wo", "Wg", "Wu", "Wd", "Wout"]


def build_kernel(nc, tc, bass_mod, mybir, TL):
    """Emit the full per-core program. TL = local timesteps (multiple of 128)."""
    from contextlib import ExitStack
    from concourse.masks import make_identity

    BT = TL // P
    dt = mybir.dt
    AF = mybir.ActivationFunctionType
    OP = mybir.AluOpType
    AX = mybir.AxisListType

    # ---------------- I/O ----------------
    def din(name, shape, dtype=None):
        return nc.dram_tensor(name, shape, dtype or dt.float32, kind="ExternalInput").ap()

    x0_t_d = din("x0_t", [H, TL])
    wshape = {"Wq": [H, H], "Wk": [H, H], "Wv": [H, H], "Wo": [H, H],
              "Wg": [H, I], "Wu": [H, I], "Wd": [I, H], "Wout": [H, V]}
    Whi = {w: din(w + "_hi", wshape[w], dt.float16) for w in WEIGHTS}
    Wlo = {w: din(w + "_lo", wshape[w], dt.float16) for w in WEIGHTS}
    E_d = din("E", [V, H])
    n1_d = din("n1", [H]); n2_d = din("n2", [H]); bout_d = din("bout", [V])
    cq_d = din("rope_cos_q", [NSTEPS, H]); sq_d = din("rope_sin_q", [NSTEPS, H])
    ck_d = din("rope_cos_k", [NSTEPS, H]); sk_d = din("rope_sin_k", [NSTEPS, H])
    toks_d = nc.dram_tensor("toks", [TL, NSTEPS], dt.int32, kind="ExternalOutput").ap()

    # ---------------- scratch DRAM ----------------
    def dscratch(name, shape):
        return nc.dram_tensor(name, shape, dt.float32, kind="Internal").ap()

    qE_raw_d = dscratch("qE_raw", [V, H])
    kE_raw_d = dscratch("kE_raw", [V, H])
    vE_d = dscratch("vE_tab", [V, H])
    qEp_d = [None] + [dscratch(f"qE_p{p}", [V, H]) for p in range(1, NSTEPS)]
    kEp_d = [None] + [dscratch(f"kE_p{p}", [V, H]) for p in range(1, NSTEPS)]
    Kc_d = dscratch("K_cache", [TL, NSTEPS, H])
    Vc_d = dscratch("V_cache", [TL, NSTEPS, H])
    cur_f_d = dscratch("cur_f", [H, TL])
    h_f_d = dscratch("h_f", [H, TL])

    ctx = ExitStack()
    with ctx:
        # -------- pools --------
        sb = ctx.enter_context(tc.tile_pool(name="sb", bufs=1))
        ps_pool = ctx.enter_context(tc.tile_pool(name="ps", bufs=1, space="PSUM"))

        def til(shape, tag, bufs, dtype=dt.float32, name=None):
            return sb.tile(shape, dtype, tag=tag, bufs=bufs, name=name or tag)

        def psum(tag="ps", bufs=5, shape=(P, 512), name=None):
            return ps_pool.tile(list(shape), dt.float32, space="PSUM", tag=tag,
                                bufs=bufs, name=name or tag)

        # -------- constants --------
        ident = til([P, P], "consts_id", 1, name="ident")
        make_identity(nc, ident[:])
        ones_m = til([1, P], "consts_ones", 1, name="ones_m")
        nc.vector.memset(ones_m[:], 1.0)
        invH = til([P, 1], "consts_invH", 1, name="invH")
        nc.vector.memset(invH[:], 1.0 / H)
        n1_sb = til([P, HKT], "consts_n1", 1, name="n1_sb")
        nc.sync.dma_start(n1_sb[:], n1_d.rearrange("(kt p) -> p kt", p=P))
        n2_sb = til([P, HKT], "consts_n2", 1, name="n2_sb")
        nc.sync.dma_start(n2_sb[:], n2_d.rearrange("(kt p) -> p kt", p=P))

        def bcast_row(row_sb, out_tile, width):
            """Broadcast [1, width] SBUF row across 128 partitions via PE."""
            bps = psum(tag="bc", bufs=1, shape=(P, 1024), name="bps")
            for nch in range(0, width, 512):
                w = min(512, width - nch)
                nc.tensor.matmul(bps[:, nch:nch + w], lhsT=ones_m[:1, :],
                                 rhs=row_sb[:1, nch:nch + w], start=True, stop=True)
            nc.scalar.copy(out_tile[:, :width], bps[:, :width])

        n1row = til([1, H], "rowbuf", 1, name="n1row")
        nc.sync.dma_start(n1row[:1, :], n1_d[None, :])
        n1b = til([P, H], "consts_n1b", 1, name="n1b")
        bcast_row(n1row, n1b, H)
        boutrow = til([1, V], "rowbuf", 1, name="boutrow")
        nc.sync.dma_start(boutrow[:1, :], bout_d[None, :])
        boutb = til([P, V], "consts_boutb", 1, name="boutb")
        bcast_row(boutrow, boutb, V)

        # ============ helpers ============
        def split_fp16(src_f32_ap, hi_ap, lo_ap, tmp_shape):
            """hi = f16(src); lo = f16((src - hi) * S).  3 elementwise passes."""
            nc.scalar.copy(hi_ap, src_f32_ap)
            diff = til(list(tmp_shape), "tdr", 6, name="spdiff")
            dv = diff[:] if list(diff.shape) == list(src_f32_ap.shape) \
                else diff[:].rearrange("p (a b) -> p a b", b=src_f32_ap.shape[-1])
            nc.vector.tensor_tensor(dv[: ], src_f32_ap, hi_ap, op=OP.subtract)
            nc.gpsimd.tensor_scalar(lo_ap, dv[:], S, None, OP.mult)

        def combine(psA, psB, out_ap, width=None):
            """out = psA + psB/S  (ACT + DVE, one PSUM operand each)."""
            w = width or TL
            t = til([P, 512], "tdr", 6, name="cmb")
            nc.scalar.mul(t[:, :w], psB[:, :w], 1.0 / S)
            nc.vector.tensor_add(out_ap, t[:, :w], psA[:, :w])

        def newton_rsqrt(var_ap, r_ap, tmp_alloc):
            s = tmp_alloc()
            nc.scalar.activation(s, var_ap, AF.Sqrt)
            y0 = tmp_alloc()
            nc.vector.reciprocal(y0, s)
            t = tmp_alloc()
            nc.vector.tensor_mul(t, y0, y0)
            nc.vector.tensor_mul(t, t, var_ap)
            nc.vector.tensor_scalar(t, t, -0.5, 1.5, OP.mult, OP.add)
            nc.vector.tensor_mul(r_ap, y0, t)

        def rms_split_from_dram(src_d, n_sb_tile, out_hi, out_lo):
            """x = src*n*rsqrt(mean(src^2)+eps), split to fp16 hi/lo pairs.
            src_d f-major [H, TL] DRAM; out_hi/out_lo [128, HKT, TL] fp16."""
            psv = psum(tag="psv", bufs=1, shape=(1, 512), name="psv")
            for kt in range(HKT):
                hkt = til([P, TL], "tdr", 6, name="rhkt")
                nc.sync.dma_start(hkt[:], src_d[kt * P:(kt + 1) * P, :])
                sqk = til([P, TL], "tdr", 6, name="rsqk")
                nc.scalar.activation(sqk[:], hkt[:], AF.Square)
                nc.tensor.matmul(psv[:1, :TL], lhsT=invH[:, :1], rhs=sqk[:],
                                 start=(kt == 0), stop=(kt == HKT - 1),
                                 skip_group_check=True)
            v_sb = til([1, TL], "sr", 3, name="v_sb")
            nc.vector.tensor_scalar(v_sb[:1, :], psv[:1, :TL], EPS, None, OP.add)
            r = til([1, TL], "sr", 3, name="r_sb")
            newton_rsqrt(v_sb[:1, :], r[:1, :],
                         lambda: til([1, TL], "sr", 3, name="nt")[:1, :])
            rb = psum(name="rbps")
            nc.tensor.matmul(rb[:, :TL], lhsT=ones_m[:1, :], rhs=r[:1, :TL],
                             start=True, stop=True)
            for kt in range(HKT):
                hkt = til([P, TL], "tdr", 6, name="rhkt2")
                nc.sync.dma_start(hkt[:], src_d[kt * P:(kt + 1) * P, :])
                xkt = til([P, TL], "tdr", 6, name="rxkt")
                nc.vector.scalar_tensor_tensor(xkt[:], hkt[:],
                                               n_sb_tile[:, kt:kt + 1], rb[:, :TL],
                                               op0=OP.mult, op1=OP.mult)
                split_fp16(xkt[:], out_hi[:, kt, :], out_lo[:, kt, :], (P, TL))

        def mm16(whi_d, wlo_d, rhs_hi, rhs_lo, drain_fn, n_mt=HKT, n_kt=HKT):
            """psA = sum_kt Whi[mt].T@rhs_hi; psB = sum (Wlo@rhs_hi + Whi@rhs_lo);
            drain_fn(mt, psA, psB)."""
            for mt in range(n_mt):
                whi_t = til([P, n_kt, P], "w2", 8, dtype=dt.float16, name="whi_t")
                nc.sync.dma_start(whi_t[:], whi_d[:, mt * P:(mt + 1) * P]
                                  .rearrange("(kt p) m -> p kt m", p=P))
                wlo_t = til([P, n_kt, P], "w2", 8, dtype=dt.float16, name="wlo_t")
                nc.sync.dma_start(wlo_t[:], wlo_d[:, mt * P:(mt + 1) * P]
                                  .rearrange("(kt p) m -> p kt m", p=P))
                psA = psum(name="psA")
                for kt in range(n_kt):
                    nc.tensor.matmul(psA[:, :TL], lhsT=whi_t[:, kt, :], rhs=rhs_hi[:, kt, :],
                                     start=(kt == 0), stop=(kt == n_kt - 1))
                psB = psum(name="psB")
                for kt in range(n_kt):
                    nc.tensor.matmul(psB[:, :TL], lhsT=wlo_t[:, kt, :], rhs=rhs_hi[:, kt, :],
                                     start=(kt == 0), stop=False)
                for kt in range(n_kt):
                    nc.tensor.matmul(psB[:, :TL], lhsT=whi_t[:, kt, :], rhs=rhs_lo[:, kt, :],
                                     start=False, stop=(kt == n_kt - 1))
                drain_fn(mt, psA, psB)

        def transpose_b2f_split(src_b_list, dst_hi, dst_lo):
            """BT batch-major [128, H] tiles -> f-major fp16 hi/lo pairs."""
            for kt in range(HKT):
                ps = psum(name="tps")
                for bt in range(BT):
                    nc.tensor.transpose(ps[:, bt * P:(bt + 1) * P],
                                        src_b_list[bt][:, kt * P:(kt + 1) * P], ident[:])
                row = til([P, 512], "tdr", 6, name="trow")
                nc.scalar.copy(row[:, :TL], ps[:, :TL])
                split_fp16(row[:, :TL], dst_hi[:, kt, :], dst_lo[:, kt, :], (P, TL))

        def transpose_b2f_to_dram(src_b, bt, dst_d):
            for g in range(2):
                ps = psum(name="tps2")
                for i4 in range(4):
                    kt = g * 4 + i4
                    nc.tensor.transpose(ps[:, i4 * P:(i4 + 1) * P],
                                        src_b[:, kt * P:(kt + 1) * P], ident[:])
                cp = til([P, 512], "tdr", 6, name="tcp")
                nc.scalar.copy(cp[:], ps[:])
                for i4 in range(4):
                    kt = g * 4 + i4
                    nc.sync.dma_start(dst_d[kt * P:(kt + 1) * P, bt * P:(bt + 1) * P],
                                      cp[:, i4 * P:(i4 + 1) * P])

        def gather_rows(table_d, tok_ap, out_tile):
            nc.gpsimd.indirect_dma_start(
                out=out_tile[:], out_offset=None, in_=table_d[:, :],
                in_offset=bass_mod.IndirectOffsetOnAxis(ap=tok_ap, axis=0))

        def rope_apply(src, cosb, sinb, out):
            """out = src*cos + swap(src)*sin' ; all [128, H]."""
            nc.vector.tensor_mul(out[:], src[:], cosb[:])
            t2 = til([P, H], "tmp", 2, name="ropet2")
            src3 = src[:].rearrange("p (h d) -> p h d", d=HD)
            sin3 = sinb[:].rearrange("p (h d) -> p h d", d=HD)
            t23 = t2[:].rearrange("p (h d) -> p h d", d=HD)
            nc.gpsimd.tensor_tensor(t23[:, :, 0:HD // 2], src3[:, :, HD // 2:HD],
                                    sin3[:, :, 0:HD // 2], op=OP.mult)
            nc.gpsimd.tensor_tensor(t23[:, :, HD // 2:HD], src3[:, :, 0:HD // 2],
                                    sin3[:, :, HD // 2:HD], op=OP.mult)
            nc.vector.tensor_add(out[:], out[:], t2[:])

        def load_rope_bcast_pair(p, ctab, stab):
            out = []
            for nm, tab in (("c", ctab), ("s", stab)):
                row = til([1, H], "rowbuf", 1, name=f"row_{nm}")
                nc.sync.dma_start(row[:1, :], tab[p:p + 1, :])
                bt_tile = til([P, H], "g4", 5, name=f"rb_{nm}")
                bcast_row(row, bt_tile, H)
                out.append(bt_tile)
            return out

        # ============ Phase A: token tables ============
        # A1: rmsnorm rows of E (v-major) -> transposed fp16 hi/lo [h, v]
        rEt_hi = til([P, HKT, H], "big16", 2, dtype=dt.float16, name="rEt_hi")
        rEt_lo = til([P, HKT, H], "big16", 2, dtype=dt.float16, name="rEt_lo")
        for vt in range(HKT):
            ev = til([P, H], "g4", 5, name="ev")
            nc.sync.dma_start(ev[:], E_d[vt * P:(vt + 1) * P, :])
            sqe = til([P, H], "tmp", 2, name="sqe")
            nc.scalar.activation(sqe[:], ev[:], AF.Square)
            ssum = til([P, 1], "s1", 3, name="ssum")
            nc.vector.tensor_reduce(ssum[:], sqe[:], axis=AX.X, op=OP.add)
            var = til([P, 1], "s1", 3, name="var")
            nc.vector.tensor_scalar(var[:], ssum[:], 1.0 / H, EPS, OP.mult, OP.add)
            r1 = til([P, 1], "s1", 3, name="r1")
            newton_rsqrt(var[:], r1[:], lambda: til([P, 1], "s1", 3, name="nt1")[:])
            rme = til([P, H], "g4", 5, name="rme")
            nc.vector.scalar_tensor_tensor(rme[:], ev[:], r1[:, :1], n1b[:],
                                           op0=OP.mult, op1=OP.mult)
            for g in range(2):
                ps = psum(name="taps")
                for i4 in range(4):
                    ht = g * 4 + i4
                    nc.tensor.transpose(ps[:, i4 * P:(i4 + 1) * P],
                                        rme[:, ht * P:(ht + 1) * P], ident[:])
                row = til([P, 512], "tdr", 6, name="tarow")
                nc.scalar.copy(row[:], ps[:])
                split_fp16(row[:].rearrange("p (a b) -> p a b", b=P),
                           rEt_hi[:, g * 4:(g + 1) * 4, vt * P:(vt + 1) * P],
                           rEt_lo[:, g * 4:(g + 1) * 4, vt * P:(vt + 1) * P],
                           (P, 512))
        # A2: raw tables = (rmsE @ W) -> DRAM (v-major rows); fp16x2 on both sides
        for w_name, dst in (("Wq", qE_raw_d), ("Wk", kE_raw_d), ("Wv", vE_d)):
            for nch in range(2):
                whi_t = til([P, HKT, 512], "w8", 2, dtype=dt.float16, name="tbl_whi")
                nc.sync.dma_start(whi_t[:], Whi[w_name][:, nch * 512:(nch + 1) * 512]
                                  .rearrange("(kt p) n -> p kt n", p=P))
                wlo_t = til([P, HKT, 512], "w8", 2, dtype=dt.float16, name="tbl_wlo")
                nc.sync.dma_start(wlo_t[:], Wlo[w_name][:, nch * 512:(nch + 1) * 512]
                                  .rearrange("(kt p) n -> p kt n", p=P))
                for mt in range(HKT):
                    psA = psum(name="tpsA")
                    for kt in range(HKT):
                        nc.tensor.matmul(psA[:], lhsT=rEt_hi[:, kt, mt * P:(mt + 1) * P],
                                         rhs=whi_t[:, kt, :], start=(kt == 0), stop=(kt == HKT - 1))
                    psB = psum(name="tpsB")
                    for kt in range(HKT):
                        nc.tensor.matmul(psB[:], lhsT=rEt_lo[:, kt, mt * P:(mt + 1) * P],
                                         rhs=whi_t[:, kt, :], start=(kt == 0), stop=False)
                    for kt in range(HKT):
                        nc.tensor.matmul(psB[:], lhsT=rEt_hi[:, kt, mt * P:(mt + 1) * P],
                                         rhs=wlo_t[:, kt, :], start=False, stop=(kt == HKT - 1))
                    ot = til([P, 512], "tdr", 6, name="tot")
                    combine(psA, psB, ot[:, :512], width=512)
                    nc.sync.dma_start(dst[mt * P:(mt + 1) * P, nch * 512:(nch + 1) * 512], ot[:])
        # A3: rope-baked tables for p=1..7
        for p in range(1, NSTEPS):
            for raw_d, dst_d, ctab, stab in ((qE_raw_d, qEp_d[p], cq_d, sq_d),
                                             (kE_raw_d, kEp_d[p], ck_d, sk_d)):
                cosb, sinb = load_rope_bcast_pair(p, ctab, stab)
                for vt in range(HKT):
                    raw = til([P, H], "g4", 5, name="rawv")
                    nc.sync.dma_start(raw[:], raw_d[vt * P:(vt + 1) * P, :])
                    outp = til([P, H], "g4", 5, name="outp")
                    rope_apply(raw, cosb, sinb, outp)
                    nc.sync.dma_start(dst_d[vt * P:(vt + 1) * P, :], outp[:])

        # ============ per-step tail: rms2 + MLP + logits + argmax ============
        def mlp_and_logits(p, tok_tiles):
            """h is in h_f_d (DRAM, f-major).  Produces tok_tiles + toks output."""
            x2h = til([P, HKT, TL], "act8", 4, dtype=dt.float16, name="x2h")
            x2l = til([P, HKT, TL], "act8", 4, dtype=dt.float16, name="x2l")
            rms_split_from_dram(h_f_d, n2_sb, x2h, x2l)
            hfh = til([P, HKT, TL], "act8", 4, dtype=dt.float16, name="hfh")
            hfl = til([P, HKT, TL], "act8", 4, dtype=dt.float16, name="hfl")
            for ihalf in range(2):
                sih = til([P, IH, TL], "big16", 2, dtype=dt.float16, name="sih")
                sil = til([P, IH, TL], "big16", 2, dtype=dt.float16, name="sil")
                for im in range(IH):
                    imt = ihalf * IH + im
                    wg_h = til([P, HKT, P], "w2", 8, dtype=dt.float16, name="wg_h")
                    nc.sync.dma_start(wg_h[:], Whi["Wg"][:, imt * P:(imt + 1) * P]
                                      .rearrange("(kt p) m -> p kt m", p=P))
                    wg_l = til([P, HKT, P], "w2", 8, dtype=dt.float16, name="wg_l")
                    nc.sync.dma_start(wg_l[:], Wlo["Wg"][:, imt * P:(imt + 1) * P]
                                      .rearrange("(kt p) m -> p kt m", p=P))
                    wu_h = til([P, HKT, P], "w2", 8, dtype=dt.float16, name="wu_h")
                    nc.sync.dma_start(wu_h[:], Whi["Wu"][:, imt * P:(imt + 1) * P]
                                      .rearrange("(kt p) m -> p kt m", p=P))
                    wu_l = til([P, HKT, P], "w2", 8, dtype=dt.float16, name="wu_l")
                    nc.sync.dma_start(wu_l[:], Wlo["Wu"][:, imt * P:(imt + 1) * P]
                                      .rearrange("(kt p) m -> p kt m", p=P))

                    def mmgrp(psA, psB, wh, wl):
                        for kt in range(HKT):
                            nc.tensor.matmul(psA[:, :TL], lhsT=wh[:, kt, :], rhs=x2h[:, kt, :],
                                             start=(kt == 0), stop=(kt == HKT - 1))
                        for kt in range(HKT):
                            nc.tensor.matmul(psB[:, :TL], lhsT=wl[:, kt, :], rhs=x2h[:, kt, :],
                                             start=(kt == 0), stop=False)
                        for kt in range(HKT):
                            nc.tensor.matmul(psB[:, :TL], lhsT=wh[:, kt, :], rhs=x2l[:, kt, :],
                                             start=False, stop=(kt == HKT - 1))
                    psgA = psum(name="psgA"); psgB = psum(name="psgB")
                    mmgrp(psgA, psgB, wg_h, wg_l)
                    psuA = psum(name="psuA"); psuB = psum(name="psuB")
                    mmgrp(psuA, psuB, wu_h, wu_l)
                    gc = til([P, TL], "tdr", 6, name="gc")
                    combine(psgA, psgB, gc[:])
                    uc = til([P, TL], "tdr", 6, name="uc")
                    combine(psuA, psuB, uc[:])
                    sg = til([P, TL], "tdr", 6, name="sg")
                    nc.scalar.activation(sg[:], gc[:], AF.Sigmoid)
                    nc.vector.tensor_mul(sg[:], sg[:], gc[:])
                    sv = til([P, TL], "tdr", 6, name="sv")
                    nc.vector.tensor_mul(sv[:], sg[:], uc[:])
                    split_fp16(sv[:], sih[:, im, :], sil[:, im, :], (P, TL))
                for hmt in range(HKT):
                    wd_h = til([P, IH, P], "w4", 4, dtype=dt.float16, name="wd_h")
                    nc.sync.dma_start(wd_h[:], Whi["Wd"][ihalf * (I // 2):(ihalf + 1) * (I // 2),
                                                         hmt * P:(hmt + 1) * P]
                                      .rearrange("(kt p) m -> p kt m", p=P))
                    wd_l = til([P, IH, P], "w4", 4, dtype=dt.float16, name="wd_l")
                    nc.sync.dma_start(wd_l[:], Wlo["Wd"][ihalf * (I // 2):(ihalf + 1) * (I // 2),
                                                         hmt * P:(hmt + 1) * P]
                                      .rearrange("(kt p) m -> p kt m", p=P))
                    psA = psum(name="psdA")
                    for kt in range(IH):
                        nc.tensor.matmul(psA[:, :TL], lhsT=wd_h[:, kt, :], rhs=sih[:, kt, :],
                                         start=(kt == 0), stop=(kt == IH - 1))
                    psB = psum(name="psdB")
                    for kt in range(IH):
                        nc.tensor.matmul(psB[:, :TL], lhsT=wd_l[:, kt, :], rhs=sih[:, kt, :],
                                         start=(kt == 0), stop=False)
                    for kt in range(IH):
                        nc.tensor.matmul(psB[:, :TL], lhsT=wd_h[:, kt, :], rhs=sil[:, kt, :],
                                         start=False, stop=(kt == IH - 1))
                    dsum = til([P, TL], "tdr", 6, name="dsum")
                    combine(psA, psB, dsum[:])
                    hfm = til([P, TL], "tdr", 6, name="hfm")
                    if ihalf == 0:
                        hres = til([P, TL], "tdr", 6, name="hres")
                        nc.sync.dma_start(hres[:], h_f_d[hmt * P:(hmt + 1) * P, :])
                        nc.vector.tensor_add(hfm[:], dsum[:], hres[:])
                        # stash partial in fp16 pair
                        split_fp16(hfm[:], hfh[:, hmt, :], hfl[:, hmt, :], (P, TL))
                    else:
                        prev = til([P, TL], "tdr", 6, name="prev")
                        nc.vector.scalar_tensor_tensor(prev[:], hfl[:, hmt, :], 1.0 / S,
                                                       hfh[:, hmt, :], op0=OP.mult, op1=OP.add)
                        nc.vector.tensor_add(hfm[:], dsum[:], prev[:])
                        split_fp16(hfm[:], hfh[:, hmt, :], hfl[:, hmt, :], (P, TL))
            # logits (batch-major) + argmax; reload Wout chunks per batch tile
            for bt in range(BT):
                lg = til([P, V], "lg", 2, name="lg")
                for nch in range(2):
                    wh = til([P, HKT, 512], "w8", 2, dtype=dt.float16, name=f"wouth{nch}")
                    nc.sync.dma_start(wh[:], Whi["Wout"][:, nch * 512:(nch + 1) * 512]
                                      .rearrange("(kt p) n -> p kt n", p=P))
                    wl = til([P, HKT, 512], "w8", 2, dtype=dt.float16, name=f"woutl{nch}")
                    nc.sync.dma_start(wl[:], Wlo["Wout"][:, nch * 512:(nch + 1) * 512]
                                      .rearrange("(kt p) n -> p kt n", p=P))
                    psA = psum(name="lgA")
                    for kt in range(HKT):
                        nc.tensor.matmul(psA[:], lhsT=hfh[:, kt, bt * P:(bt + 1) * P],
                                         rhs=wh[:, kt, :], start=(kt == 0), stop=(kt == HKT - 1))
                    psB = psum(name="lgB")
                    for kt in range(HKT):
                        nc.tensor.matmul(psB[:], lhsT=hfl[:, kt, bt * P:(bt + 1) * P],
                                         rhs=wh[:, kt, :], start=(kt == 0), stop=False)
                    for kt in range(HKT):
                        nc.tensor.matmul(psB[:], lhsT=hfh[:, kt, bt * P:(bt + 1) * P],
                                         rhs=wl[:, kt, :], start=False, stop=(kt == HKT - 1))
                    t = til([P, 512], "tdr", 6, name="lgt")
                    nc.scalar.mul(t[:], psB[:], 1.0 / S)
                    nc.vector.tensor_add(t[:], t[:], psA[:])
                    nc.vector.tensor_add(lg[:, nch * 512:(nch + 1) * 512], t[:],
                                         boutb[:, nch * 512:(nch + 1) * 512])
                mx = til([P, 8], "mx", 2, name="mx")
                nc.vector.max(mx[:], lg[:])
                idx = til([P, 8], "idx", 2, dtype=dt.uint32, name="idx")
                nc.vector.max_index(idx[:], mx[:], lg[:])
                tok = til([P, 1], "tok", 2 * BT + 1, dtype=dt.uint32, name="tok")
                nc.vector.tensor_copy(tok[:], idx[:, 0:1])
                tok_tiles[bt] = tok
                nc.sync.dma_start(toks_d[bt * P:(bt + 1) * P, p:p + 1],
                                  tok[:].bitcast(dt.int32))

        # ============ Phase B: step 0 ============
        x1h = til([P, HKT, TL], "act8", 4, dtype=dt.float16, name="x1h")
        x1l = til([P, HKT, TL], "act8", 4, dtype=dt.float16, name="x1l")
        rms_split_from_dram(x0_t_d, n1_sb, x1h, x1l)
        # k0 / v0: fp16x2 matmuls; drain transposed straight to the DRAM cache
        v0h = til([P, HKT, TL], "act8", 4, dtype=dt.float16, name="v0h")
        v0l = til([P, HKT, TL], "act8", 4, dtype=dt.float16, name="v0l")
        for w_name, is_v in (("Wk", False), ("Wv", True)):
            dst = Vc_d if is_v else Kc_d

            def drain_kv(mt, psA, psB, dst=dst, is_v=is_v):
                row = til([P, TL], "tdr", 6, name="kvrow")
                combine(psA, psB, row[:])
                if is_v:
                    split_fp16(row[:], v0h[:, mt, :], v0l[:, mt, :], (P, TL))
                tp = psum(name="kvtps")
                for bt in range(BT):
                    nc.tensor.transpose(tp[:, bt * P:(bt + 1) * P],
                                        row[:, bt * P:(bt + 1) * P], ident[:])
                row2 = til([P, TL], "tdr", 6, name="kvrow2")
                nc.vector.tensor_copy(row2[:], tp[:, :TL])
                for bt in range(BT):
                    nc.sync.dma_start(dst[bt * P:(bt + 1) * P, 0, mt * P:(mt + 1) * P],
                                      row2[:, bt * P:(bt + 1) * P])
            mm16(Whi[w_name], Wlo[w_name], x1h, x1l, drain_kv)

        # h = x0 + v0 @ Wo  (attention at step 0 is identity: ctx = v0)
        def drain_h_step0(mt, psA, psB):
            hs = til([P, TL], "tdr", 6, name="hs0")
            combine(psA, psB, hs[:])
            x0kt = til([P, TL], "tdr", 6, name="x0kt")
            nc.sync.dma_start(x0kt[:], x0_t_d[mt * P:(mt + 1) * P, :])
            nc.vector.tensor_add(hs[:], hs[:], x0kt[:])
            nc.sync.dma_start(h_f_d[mt * P:(mt + 1) * P, :], hs[:])
        mm16(Whi["Wo"], Wlo["Wo"], v0h, v0l, drain_h_step0)
        tok_tiles = [None] * BT
        mlp_and_logits(0, tok_tiles)

        # ============ Phase C: steps 1..7 ============
        for p in range(1, NSTEPS):
            cxh = til([P, HKT, TL], "act8", 4, dtype=dt.float16, name="cxh")
            cxl = til([P, HKT, TL], "act8", 4, dtype=dt.float16, name="cxl")
            ctx_b = [None] * BT
            for bt in range(BT):
                tokap = tok_tiles[bt][:, :1]
                cb = til([P, H], "g4", 5, name="cb")
                gather_rows(E_d, tokap, cb)
                transpose_b2f_to_dram(cb, bt, cur_f_d)
                qp = til([P, H], "g4", 5, name="qp")
                gather_rows(qEp_d[p], tokap, qp)
                kp = til([P, H], "g4", 5, name="kp")
                gather_rows(kEp_d[p], tokap, kp)
                vp = til([P, H], "g4", 5, name="vp")
                gather_rows(vE_d, tokap, vp)
                nc.sync.dma_start(Kc_d[bt * P:(bt + 1) * P, p, :], kp[:])
                nc.sync.dma_start(Vc_d[bt * P:(bt + 1) * P, p, :], vp[:])
                # ---- attention ----
                s_sb = til([P, P], "s", 2, name="s_sb")
                for j in range(p + 1):
                    if j < p:
                        Kj = til([P, H], "kv", 2, name="Kj")
                        nc.sync.dma_start(Kj[:], Kc_d[bt * P:(bt + 1) * P, j, :])
                    else:
                        Kj = kp
                    tmp = til([P, H], "tmp", 2, name="atmp")
                    nc.vector.tensor_mul(tmp[:], Kj[:], qp[:])
                    nc.vector.tensor_reduce(
                        s_sb[:, j * NH:(j + 1) * NH],
                        tmp[:].rearrange("p (h d) -> p h d", d=HD),
                        axis=AX.X, op=OP.add)
                m16 = til([P, NH], "m16", 4, name="m16")
                nc.vector.tensor_reduce(
                    m16[:], s_sb[:, :(p + 1) * NH].rearrange("p (j h) -> p h j", h=NH),
                    axis=AX.X, op=OP.max)
                e_sb = til([P, P], "s", 2, name="e_sb")
                nc.vector.tensor_tensor(
                    e_sb[:, :(p + 1) * NH], s_sb[:, :(p + 1) * NH],
                    m16[:].unsqueeze(1).to_broadcast([P, p + 1, NH]),
                    op=OP.subtract)
                nc.scalar.activation(e_sb[:, :(p + 1) * NH], e_sb[:, :(p + 1) * NH], AF.Exp)
                z16 = til([P, NH], "m16", 4, name="z16")
                nc.vector.tensor_reduce(
                    z16[:], e_sb[:, :(p + 1) * NH].rearrange("p (j h) -> p h j", h=NH),
                    axis=AX.X, op=OP.add)
                rz = til([P, NH], "m16", 4, name="rz")
                nc.vector.reciprocal(rz[:], z16[:])
                nc.vector.tensor_tensor(
                    e_sb[:, :(p + 1) * NH], e_sb[:, :(p + 1) * NH],
                    rz[:].unsqueeze(1).to_broadcast([P, p + 1, NH]),
                    op=OP.mult)
                cx = til([P, H], "cx", BT, name="cx")
                for j in range(p + 1):
                    if j < p:
                        Vj = til([P, H], "kv", 2, name="Vj")
                        nc.sync.dma_start(Vj[:], Vc_d[bt * P:(bt + 1) * P, j, :])
                    else:
                        Vj = vp
                    aj = e_sb[:, j * NH:(j + 1) * NH].unsqueeze(2).to_broadcast([P, NH, HD])
                    if j == 0:
                        nc.vector.tensor_tensor(
                            cx[:].rearrange("p (h d) -> p h d", d=HD),
                            Vj[:].rearrange("p (h d) -> p h d", d=HD), aj, op=OP.mult)
                    else:
                        tmp = til([P, H], "tmp", 2, name="atmp2")
                        nc.vector.tensor_tensor(
                            tmp[:].rearrange("p (h d) -> p h d", d=HD),
                            Vj[:].rearrange("p (h d) -> p h d", d=HD), aj, op=OP.mult)
                        nc.vector.tensor_add(cx[:], cx[:], tmp[:])
                ctx_b[bt] = cx
            transpose_b2f_split(ctx_b, cxh, cxl)

            # h = cur + ctx @ Wo
            def drain_h(mt, psA, psB):
                hs = til([P, TL], "tdr", 6, name="hs")
                combine(psA, psB, hs[:])
                ckt = til([P, TL], "tdr", 6, name="ckt")
                nc.sync.dma_start(ckt[:], cur_f_d[mt * P:(mt + 1) * P, :])
                nc.vector.tensor_add(hs[:], hs[:], ckt[:])
                nc.sync.dma_start(h_f_d[mt * P:(mt + 1) * P, :], hs[:])
            mm16(Whi["Wo"], Wlo["Wo"], cxh, cxl, drain_h)
            mlp_and_logits(p, tok_tiles)

    return nc


_CACHED = {}


def _build(TL):
    if TL in _CACHED:
        return _CACHED[TL]
    import concourse.bass as bass
    import concourse.tile as tile
    from concourse import bacc, mybir
    nc = bacc.Bacc("TRN2", target_bir_lowering=False, debug=False, num_devices=N_CORES)
    with tile.TileContext(nc) as tc:
        build_kernel(nc, tc, bass, mybir, TL)
    nc.compile()
    _CACHED[TL] = nc
    return nc


def make_in_maps(inputs, n_cores=N_CORES, TL=None):
    """Shard/augment the full inputs into per-core in_maps."""
    x0 = np.ascontiguousarray(np.asarray(inputs["chunk_hidden_states"], dtype=np.float32)[0])  # [T, H]
    T = x0.shape[0]
    if TL is None:
        TL = T // n_cores
    cq, sq, ck, sk = _rope_tables()
    shared = {
        "E": np.asarray(inputs["E"], np.float32),
        "n1": np.asarray(inputs["n1"], np.float32), "n2": np.asarray(inputs["n2"], np.float32),
        "bout": np.asarray(inputs["bout"], np.float32),
        "rope_cos_q": cq, "rope_sin_q": sq, "rope_cos_k": ck, "rope_sin_k": sk,
    }
    for w in WEIGHTS:
        wf = np.asarray(inputs[w], np.float32)
        hi = wf.astype(np.float16)
        lo = ((wf - hi.astype(np.float32)) * np.float32(S)).astype(np.float16)
        shared[w + "_hi"] = hi
        shared[w + "_lo"] = lo
    in_maps = []
    for c in range(n_cores):
        m = dict(shared)
        m["x0_t"] = np.ascontiguousarray(x0[c * TL:(c + 1) * TL, :].T)
        in_maps.append(m)
    return in_maps, TL


def kernel(**inputs):
    from concourse.bass_utils import run_bass_kernel_spmd
    in_maps, TL = make_in_maps(inputs)
    nc = _build(TL)
    res = run_bass_kernel_spmd(nc, in_maps, core_ids=list(range(N_CORES)))
    toks = np.concatenate([r["toks"] for r in res.results], axis=0)  # [T, 8]
    return toks.astype(np.int32)


# revision 14
# speedup vs baseline: 1.1615x; 1.1380x over previous
"""Trainium2 Bass kernel for nn_AutoregressivePredictor.

Strategy
--------
Data-parallel over the 4096 independent timesteps: 8 cores x 512 timesteps.
Each timestep runs an 8-step autoregressive chain through one decoder layer.

Key algorithmic points:
  * KV caching: buffer row j never changes after it is written, so each step
    only computes q/k/v for the NEW row and attends over cached K/V.
  * Token tables: for steps p>=1 the new row is E[tok], so
    q/k/v = rope_p((rmsnorm(E)*n1 @ W)[tok]).  We precompute, once, on device:
    vE = rmsnorm(E)n1@Wv and rope-baked tables qE_p / kE_p for p=1..7.
    Per step this replaces three [512,1024]x[1024,1024] matmuls + rope with
    four row-gathers.
  * fp16 hi/lo split matmuls (3 passes: hi@hi + 2^-11*(hi@lo + lo@hi)):
    measured max error 5.3e-7 (same as the PE's fp32 mode) at 3 cycles/row
    instead of fp32's 4.  Raw fp32r/bf16/fp16 were measured far too imprecise
    for the ~2e-6 top-2 logit margins.  Weights are split host-side; the
    activation operand is split on device (3 cheap elementwise passes).
  * Activations flow feature-major [feat, batch]: every matmul uses the
    weight as stationary lhsT, so no transposes in the main chain.  Only ctx
    and cur (which arrive batch-major from attention/gathers) are transposed
    via the PE.
  * rsqrt for rmsnorm gets one Newton refinement (ACT Sqrt table is ~7e-6).

Self-contained: hardcodes all shapes; creates its own Bass program.
"""

import numpy as np

P = 128
H = 1024
NH = 16
HD = 64
I = 4096
V = 1024
T_FULL = 4096
NSTEPS = 8
N_CORES = 8
EPS = 1e-6
THETA = 10000.0
HKT = H // P          # 8 k-tiles over hidden dim
IKT = I // P          # 32 k-tiles over intermediate dim
IH = IKT // 2         # 16 i-tiles per half
S = 2048.0            # lo-part scale (2^11) keeps fp16 lo parts normal


def _rope_tables():
    """Position-expanded cos/sin tables [NSTEPS, H], fp32, matching reference.

    For head-dim layout [h*64+d]:
      out[d] = x[d]*cos[d] + rot(x)[d]*sin[d]
      rot(x)[d] = -x[d+32] (d<32), x[d-32] (d>=32)
    We implement rot(x)*sin as swap(x)*sin' with
      sin'[d] = -sin_half[d] (d<32), +sin_half[d-32] (d>=32)
    cos[d] = cos_half[d % 32].
    The q-side tables additionally fold in the 1/sqrt(HD) score scale.
    """
    inv_freq = (1.0 / (THETA ** (np.arange(0, HD, 2, dtype=np.float32) / np.float32(HD)))).astype(np.float32)
    cos_t = np.zeros((NSTEPS, H), np.float32)
    sin_t = np.zeros((NSTEPS, H), np.float32)
    for p in range(NSTEPS):
        ang = (np.float32(p) * inv_freq).astype(np.float32)  # [32]
        ch = np.cos(ang).astype(np.float32)
        sh = np.sin(ang).astype(np.float32)
        cos64 = np.concatenate([ch, ch])                      # [64]
        sinp64 = np.concatenate([-sh, sh])                    # sign-folded
        cos_t[p] = np.tile(cos64, NH)
        sin_t[p] = np.tile(sinp64, NH)
    scale = np.float32(1.0 / np.sqrt(np.float32(HD)))
    return cos_t * scale, sin_t * scale, cos_t, sin_t


WEIGHTS = ["Wq", "Wk", "Wv", "Wo", "Wg", "Wu", "Wd", "Wout"]


def build_kernel(nc, tc, bass_mod, mybir, TL):
    """Emit the full per-core program. TL = local timesteps (multiple of 128)."""
    from contextlib import ExitStack
    from concourse.masks import make_identity

    BT = TL // P
    dt = mybir.dt
    AF = mybir.ActivationFunctionType
    OP = mybir.AluOpType
    AX = mybir.AxisListType

    # ---------------- I/O ----------------
    def din(name, shape, dtype=None):
        return nc.dram_tensor(name, shape, dtype or dt.float32, kind="ExternalInput").ap()

    x0_t_d = din("x0_t", [H, TL])
    wshape = {"Wq": [H, H], "Wk": [H, H], "Wv": [H, H], "Wo": [H, H],
              "Wg": [H, I], "Wu": [H, I], "Wd": [I, H], "Wout": [H, V]}
    Whi = {w: din(w + "_hi", wshape[w], dt.float16) for w in WEIGHTS}
    Wlo = {w: din(w + "_lo", wshape[w], dt.float16) for w in WEIGHTS}
    E_d = din("E", [V, H])
    n1_d = din("n1", [H]); n2_d = din("n2", [H]); bout_d = din("bout", [V])
    cq_d = din("rope_cos_q", [NSTEPS, H]); sq_d = din("rope_sin_q", [NSTEPS, H])
    ck_d = din("rope_cos_k", [NSTEPS, H]); sk_d = din("rope_sin_k", [NSTEPS, H])
    toks_d = nc.dram_tensor("toks", [TL, NSTEPS], dt.int32, kind="ExternalOutput").ap()

    # ---------------- scratch DRAM ----------------
    def dscratch(name, shape):
        return nc.dram_tensor(name, shape, dt.float32, kind="Internal").ap()

    qE_raw_d = dscratch("qE_raw", [V, H])
    kE_raw_d = dscratch("kE_raw", [V, H])
    vE_d = dscratch("vE_tab", [V, H])
    qEp_d = [None] + [dscratch(f"qE_p{p}", [V, H]) for p in range(1, NSTEPS)]
    kEp_d = [None] + [dscratch(f"kE_p{p}", [V, H]) for p in range(1, NSTEPS)]
    Kc_d = dscratch("K_cache", [TL, NSTEPS, H])
    Vc_d = dscratch("V_cache", [TL, NSTEPS, H])
    cur_f_d = dscratch("cur_f", [H, TL])
    h_f_d = dscratch("h_f", [H, TL])

    ctx = ExitStack()
    with ctx:
        # -------- pools --------
        sb = ctx.enter_context(tc.tile_pool(name="sb", bufs=1))
        ps_pool = ctx.enter_context(tc.tile_pool(name="ps", bufs=1, space="PSUM"))

        def til(shape, tag, bufs, dtype=dt.float32, name=None):
            return sb.tile(shape, dtype, tag=tag, bufs=bufs, name=name or tag)

        def psum(tag="ps", bufs=5, shape=(P, 512), name=None):
            return ps_pool.tile(list(shape), dt.float32, space="PSUM", tag=tag,
                                bufs=bufs, name=name or tag)

        # -------- constants --------
        ident = til([P, P], "consts_id", 1, name="ident")
        make_identity(nc, ident[:])
        ones_m = til([1, P], "consts_ones", 1, name="ones_m")
        nc.vector.memset(ones_m[:], 1.0)
        invH = til([P, 1], "consts_invH", 1, name="invH")
        nc.vector.memset(invH[:], 1.0 / H)
        n1_sb = til([P, HKT], "consts_n1", 1, name="n1_sb")
        nc.sync.dma_start(n1_sb[:], n1_d.rearrange("(kt p) -> p kt", p=P))
        n2_sb = til([P, HKT], "consts_n2", 1, name="n2_sb")
        nc.sync.dma_start(n2_sb[:], n2_d.rearrange("(kt p) -> p kt", p=P))

        def bcast_row(row_sb, out_tile, width):
            """Broadcast [1, width] SBUF row across 128 partitions via PE."""
            bps = psum(tag="bc", bufs=1, shape=(P, 1024), name="bps")
            for nch in range(0, width, 512):
                w = min(512, width - nch)
                nc.tensor.matmul(bps[:, nch:nch + w], lhsT=ones_m[:1, :],
                                 rhs=row_sb[:1, nch:nch + w], start=True, stop=True)
            nc.scalar.copy(out_tile[:, :width], bps[:, :width])

        n1row = til([1, H], "rowbuf", 1, name="n1row")
        nc.sync.dma_start(n1row[:1, :], n1_d[None, :])
        n1b = til([P, H], "consts_n1b", 1, name="n1b")
        bcast_row(n1row, n1b, H)
        boutrow = til([1, V], "rowbuf", 1, name="boutrow")
        nc.sync.dma_start(boutrow[:1, :], bout_d[None, :])
        boutb = til([P, V], "consts_boutb", 1, name="boutb")
        bcast_row(boutrow, boutb, V)

        # ============ helpers ============
        def split_fp16(src_f32_ap, hi_ap, lo_ap, tmp_shape):
            """hi = f16(src); lo = f16((src - hi) * S).  3 elementwise passes."""
            nc.scalar.copy(hi_ap, src_f32_ap)
            diff = til(list(tmp_shape), "tdr", 6, name="spdiff")
            dv = diff[:] if list(diff.shape) == list(src_f32_ap.shape) \
                else diff[:].rearrange("p (a b) -> p a b", b=src_f32_ap.shape[-1])
            nc.vector.tensor_tensor(dv[: ], src_f32_ap, hi_ap, op=OP.subtract)
            nc.scalar.mul(lo_ap, dv[:], S)

        def combine(psA, psB, out_ap, width=None):
            """out = psA + psB/S  (ACT + DVE, one PSUM operand each)."""
            w = width or TL
            t = til([P, 512], "tdr", 6, name="cmb")
            nc.scalar.mul(t[:, :w], psB[:, :w], 1.0 / S)
            nc.vector.tensor_add(out_ap, t[:, :w], psA[:, :w])

        def newton_rsqrt(var_ap, r_ap, tmp_alloc):
            s = tmp_alloc()
            nc.scalar.activation(s, var_ap, AF.Sqrt)
            y0 = tmp_alloc()
            nc.vector.reciprocal(y0, s)
            t = tmp_alloc()
            nc.vector.tensor_mul(t, y0, y0)
            nc.vector.tensor_mul(t, t, var_ap)
            nc.vector.tensor_scalar(t, t, -0.5, 1.5, OP.mult, OP.add)
            nc.vector.tensor_mul(r_ap, y0, t)

        def rms_split_from_dram(src_d, n_sb_tile, out_hi, out_lo):
            """x = src*n*rsqrt(mean(src^2)+eps), split to fp16 hi/lo pairs.
            src_d f-major [H, TL] DRAM; out_hi/out_lo [128, HKT, TL] fp16."""
            psv = psum(tag="psv", bufs=1, shape=(1, 512), name="psv")
            for kt in range(HKT):
                hkt = til([P, TL], "tdr", 6, name="rhkt")
                nc.sync.dma_start(hkt[:], src_d[kt * P:(kt + 1) * P, :])
                sqk = til([P, TL], "tdr", 6, name="rsqk")
                nc.scalar.activation(sqk[:], hkt[:], AF.Square)
                nc.tensor.matmul(psv[:1, :TL], lhsT=invH[:, :1], rhs=sqk[:],
                                 start=(kt == 0), stop=(kt == HKT - 1),
                                 skip_group_check=True)
            v_sb = til([1, TL], "sr", 3, name="v_sb")
            nc.vector.tensor_scalar(v_sb[:1, :], psv[:1, :TL], EPS, None, OP.add)
            r = til([1, TL], "sr", 3, name="r_sb")
            newton_rsqrt(v_sb[:1, :], r[:1, :],
                         lambda: til([1, TL], "sr", 3, name="nt")[:1, :])
            rb = psum(name="rbps")
            nc.tensor.matmul(rb[:, :TL], lhsT=ones_m[:1, :], rhs=r[:1, :TL],
                             start=True, stop=True)
            for kt in range(HKT):
                hkt = til([P, TL], "tdr", 6, name="rhkt2")
                nc.sync.dma_start(hkt[:], src_d[kt * P:(kt + 1) * P, :])
                xkt = til([P, TL], "tdr", 6, name="rxkt")
                nc.vector.scalar_tensor_tensor(xkt[:], hkt[:],
                                               n_sb_tile[:, kt:kt + 1], rb[:, :TL],
                                               op0=OP.mult, op1=OP.mult)
                split_fp16(xkt[:], out_hi[:, kt, :], out_lo[:, kt, :], (P, TL))

        def mm16(whi_d, wlo_d, rhs_hi, rhs_lo, drain_fn, n_mt=HKT, n_kt=HKT):
            """psA = sum_kt Whi[mt].T@rhs_hi; psB = sum (Wlo@rhs_hi + Whi@rhs_lo);
            drain_fn(mt, psA, psB)."""
            for mt in range(n_mt):
                whi_t = til([P, n_kt, P], "w2", 8, dtype=dt.float16, name="whi_t")
                nc.sync.dma_start(whi_t[:], whi_d[:, mt * P:(mt + 1) * P]
                                  .rearrange("(kt p) m -> p kt m", p=P))
                wlo_t = til([P, n_kt, P], "w2", 8, dtype=dt.float16, name="wlo_t")
                nc.sync.dma_start(wlo_t[:], wlo_d[:, mt * P:(mt + 1) * P]
                                  .rearrange("(kt p) m -> p kt m", p=P))
                psA = psum(name="psA")
                for kt in range(n_kt):
                    nc.tensor.matmul(psA[:, :TL], lhsT=whi_t[:, kt, :], rhs=rhs_hi[:, kt, :],
                                     start=(kt == 0), stop=(kt == n_kt - 1))
                psB = psum(name="psB")
                for kt in range(n_kt):
                    nc.tensor.matmul(psB[:, :TL], lhsT=wlo_t[:, kt, :], rhs=rhs_hi[:, kt, :],
                                     start=(kt == 0), stop=False)
                for kt in range(n_kt):
                    nc.tensor.matmul(psB[:, :TL], lhsT=whi_t[:, kt, :], rhs=rhs_lo[:, kt, :],
                                     start=False, stop=(kt == n_kt - 1))
                drain_fn(mt, psA, psB)

        def transpose_b2f_split(src_b_list, dst_hi, dst_lo):
            """BT batch-major [128, H] tiles -> f-major fp16 hi/lo pairs."""
            for kt in range(HKT):
                ps = psum(name="tps")
                for bt in range(BT):
                    nc.tensor.transpose(ps[:, bt * P:(bt + 1) * P],
                                        src_b_list[bt][:, kt * P:(kt + 1) * P], ident[:])
                row = til([P, 512], "tdr", 6, name="trow")
                nc.scalar.copy(row[:, :TL], ps[:, :TL])
                split_fp16(row[:, :TL], dst_hi[:, kt, :], dst_lo[:, kt, :], (P, TL))

        def transpose_b2f_to_dram(src_b, bt, dst_d):
            for g in range(2):
                ps = psum(name="tps2")
                for i4 in range(4):
                    kt = g * 4 + i4
                    nc.tensor.transpose(ps[:, i4 * P:(i4 + 1) * P],
                                        src_b[:, kt * P:(kt + 1) * P], ident[:])
                cp = til([P, 512], "tdr", 6, name="tcp")
                nc.scalar.copy(cp[:], ps[:])
                for i4 in range(4):
                    kt = g * 4 + i4
                    nc.sync.dma_start(dst_d[kt * P:(kt + 1) * P, bt * P:(bt + 1) * P],
                                      cp[:, i4 * P:(i4 + 1) * P])

        def gather_rows(table_d, tok_ap, out_tile):
            nc.gpsimd.indirect_dma_start(
                out=out_tile[:], out_offset=None, in_=table_d[:, :],
                in_offset=bass_mod.IndirectOffsetOnAxis(ap=tok_ap, axis=0))

        def rope_apply(src, cosb, sinb, out):
            """out = src*cos + swap(src)*sin' ; all [128, H]."""
            nc.vector.tensor_mul(out[:], src[:], cosb[:])
            t2 = til([P, H], "tmp", 2, name="ropet2")
            src3 = src[:].rearrange("p (h d) -> p h d", d=HD)
            sin3 = sinb[:].rearrange("p (h d) -> p h d", d=HD)
            t23 = t2[:].rearrange("p (h d) -> p h d", d=HD)
            nc.vector.tensor_tensor(t23[:, :, 0:HD // 2], src3[:, :, HD // 2:HD],
                                    sin3[:, :, 0:HD // 2], op=OP.mult)
            nc.vector.tensor_tensor(t23[:, :, HD // 2:HD], src3[:, :, 0:HD // 2],
                                    sin3[:, :, HD // 2:HD], op=OP.mult)
            nc.vector.tensor_add(out[:], out[:], t2[:])

        def load_rope_bcast_pair(p, ctab, stab):
            out = []
            for nm, tab in (("c", ctab), ("s", stab)):
                row = til([1, H], "rowbuf", 1, name=f"row_{nm}")
                nc.sync.dma_start(row[:1, :], tab[p:p + 1, :])
                bt_tile = til([P, H], "g4", 5, name=f"rb_{nm}")
                bcast_row(row, bt_tile, H)
                out.append(bt_tile)
            return out

        # ============ Phase A: token tables ============
        # A1: rmsnorm rows of E (v-major) -> transposed fp16 hi/lo [h, v]
        rEt_hi = til([P, HKT, H], "big16", 2, dtype=dt.float16, name="rEt_hi")
        rEt_lo = til([P, HKT, H], "big16", 2, dtype=dt.float16, name="rEt_lo")
        for vt in range(HKT):
            ev = til([P, H], "g4", 5, name="ev")
            nc.sync.dma_start(ev[:], E_d[vt * P:(vt + 1) * P, :])
            sqe = til([P, H], "tmp", 2, name="sqe")
            nc.scalar.activation(sqe[:], ev[:], AF.Square)
            ssum = til([P, 1], "s1", 3, name="ssum")
            nc.vector.tensor_reduce(ssum[:], sqe[:], axis=AX.X, op=OP.add)
            var = til([P, 1], "s1", 3, name="var")
            nc.vector.tensor_scalar(var[:], ssum[:], 1.0 / H, EPS, OP.mult, OP.add)
            r1 = til([P, 1], "s1", 3, name="r1")
            newton_rsqrt(var[:], r1[:], lambda: til([P, 1], "s1", 3, name="nt1")[:])
            rme = til([P, H], "g4", 5, name="rme")
            nc.vector.scalar_tensor_tensor(rme[:], ev[:], r1[:, :1], n1b[:],
                                           op0=OP.mult, op1=OP.mult)
            for g in range(2):
                ps = psum(name="taps")
                for i4 in range(4):
                    ht = g * 4 + i4
                    nc.tensor.transpose(ps[:, i4 * P:(i4 + 1) * P],
                                        rme[:, ht * P:(ht + 1) * P], ident[:])
                row = til([P, 512], "tdr", 6, name="tarow")
                nc.scalar.copy(row[:], ps[:])
                split_fp16(row[:].rearrange("p (a b) -> p a b", b=P),
                           rEt_hi[:, g * 4:(g + 1) * 4, vt * P:(vt + 1) * P],
                           rEt_lo[:, g * 4:(g + 1) * 4, vt * P:(vt + 1) * P],
                           (P, 512))
        # A2: raw tables = (rmsE @ W) -> DRAM (v-major rows); fp16x2 on both sides
        for w_name, dst in (("Wq", qE_raw_d), ("Wk", kE_raw_d), ("Wv", vE_d)):
            for nch in range(2):
                whi_t = til([P, HKT, 512], "w8", 2, dtype=dt.float16, name="tbl_whi")
                nc.sync.dma_start(whi_t[:], Whi[w_name][:, nch * 512:(nch + 1) * 512]
                                  .rearrange("(kt p) n -> p kt n", p=P))
                wlo_t = til([P, HKT, 512], "w8", 2, dtype=dt.float16, name="tbl_wlo")
                nc.sync.dma_start(wlo_t[:], Wlo[w_name][:, nch * 512:(nch + 1) * 512]
                                  .rearrange("(kt p) n -> p kt n", p=P))
                for mt in range(HKT):
                    psA = psum(name="tpsA")
                    for kt in range(HKT):
                        nc.tensor.matmul(psA[:], lhsT=rEt_hi[:, kt, mt * P:(mt + 1) * P],
                                         rhs=whi_t[:, kt, :], start=(kt == 0), stop=(kt == HKT - 1))
                    psB = psum(name="tpsB")
                    for kt in range(HKT):
                        nc.tensor.matmul(psB[:], lhsT=rEt_lo[:, kt, mt * P:(mt + 1) * P],
                                         rhs=whi_t[:, kt, :], start=(kt == 0), stop=False)
                    for kt in range(HKT):
                        nc.tensor.matmul(psB[:], lhsT=rEt_hi[:, kt, mt * P:(mt + 1) * P],
                                         rhs=wlo_t[:, kt, :], start=False, stop=(kt == HKT - 1))
                    ot = til([P, 512], "tdr", 6, name="tot")
                    combine(psA, psB, ot[:, :512], width=512)
                    nc.sync.dma_start(dst[mt * P:(mt + 1) * P, nch * 512:(nch + 1) * 512], ot[:])
        # A3: rope-baked tables for p=1..7
        for p in range(1, NSTEPS):
            for raw_d, dst_d, ctab, stab in ((qE_raw_d, qEp_d[p], cq_d, sq_d),
                                             (kE_raw_d, kEp_d[p], ck_d, sk_d)):
                cosb, sinb = load_rope_bcast_pair(p, ctab, stab)
                for vt in range(HKT):
                    raw = til([P, H], "g4", 5, name="rawv")
                    nc.sync.dma_start(raw[:], raw_d[vt * P:(vt + 1) * P, :])
                    outp = til([P, H], "g4", 5, name="outp")
                    rope_apply(raw, cosb, sinb, outp)
                    nc.sync.dma_start(dst_d[vt * P:(vt + 1) * P, :], outp[:])

        # ============ per-step tail: rms2 + MLP + logits + argmax ============
        def mlp_and_logits(p, tok_tiles):
            """h is in h_f_d (DRAM, f-major).  Produces tok_tiles + toks output."""
            x2h = til([P, HKT, TL], "act8", 4, dtype=dt.float16, name="x2h")
            x2l = til([P, HKT, TL], "act8", 4, dtype=dt.float16, name="x2l")
            rms_split_from_dram(h_f_d, n2_sb, x2h, x2l)
            hfh = til([P, HKT, TL], "act8", 4, dtype=dt.float16, name="hfh")
            hfl = til([P, HKT, TL], "act8", 4, dtype=dt.float16, name="hfl")
            for ihalf in range(2):
                sih = til([P, IH, TL], "big16", 2, dtype=dt.float16, name="sih")
                sil = til([P, IH, TL], "big16", 2, dtype=dt.float16, name="sil")
                for im in range(IH):
                    imt = ihalf * IH + im
                    wg_h = til([P, HKT, P], "w2", 8, dtype=dt.float16, name="wg_h")
                    nc.sync.dma_start(wg_h[:], Whi["Wg"][:, imt * P:(imt + 1) * P]
                                      .rearrange("(kt p) m -> p kt m", p=P))
                    wg_l = til([P, HKT, P], "w2", 8, dtype=dt.float16, name="wg_l")
                    nc.sync.dma_start(wg_l[:], Wlo["Wg"][:, imt * P:(imt + 1) * P]
                                      .rearrange("(kt p) m -> p kt m", p=P))
                    wu_h = til([P, HKT, P], "w2", 8, dtype=dt.float16, name="wu_h")
                    nc.sync.dma_start(wu_h[:], Whi["Wu"][:, imt * P:(imt + 1) * P]
                                      .rearrange("(kt p) m -> p kt m", p=P))
                    wu_l = til([P, HKT, P], "w2", 8, dtype=dt.float16, name="wu_l")
                    nc.sync.dma_start(wu_l[:], Wlo["Wu"][:, imt * P:(imt + 1) * P]
                                      .rearrange("(kt p) m -> p kt m", p=P))

                    def mmgrp(psA, psB, wh, wl):
                        for kt in range(HKT):
                            nc.tensor.matmul(psA[:, :TL], lhsT=wh[:, kt, :], rhs=x2h[:, kt, :],
                                             start=(kt == 0), stop=(kt == HKT - 1))
                        for kt in range(HKT):
                            nc.tensor.matmul(psB[:, :TL], lhsT=wl[:, kt, :], rhs=x2h[:, kt, :],
                                             start=(kt == 0), stop=False)
                        for kt in range(HKT):
                            nc.tensor.matmul(psB[:, :TL], lhsT=wh[:, kt, :], rhs=x2l[:, kt, :],
                                             start=False, stop=(kt == HKT - 1))
                    psgA = psum(name="psgA"); psgB = psum(name="psgB")
                    mmgrp(psgA, psgB, wg_h, wg_l)
                    psuA = psum(name="psuA"); psuB = psum(name="psuB")
                    mmgrp(psuA, psuB, wu_h, wu_l)
                    gc = til([P, TL], "tdr", 6, name="gc")
                    combine(psgA, psgB, gc[:])
                    uc = til([P, TL], "tdr", 6, name="uc")
                    combine(psuA, psuB, uc[:])
                    sg = til([P, TL], "tdr", 6, name="sg")
                    nc.scalar.activation(sg[:], gc[:], AF.Sigmoid)
                    nc.vector.tensor_mul(sg[:], sg[:], gc[:])
                    sv = til([P, TL], "tdr", 6, name="sv")
                    nc.vector.tensor_mul(sv[:], sg[:], uc[:])
                    split_fp16(sv[:], sih[:, im, :], sil[:, im, :], (P, TL))
                for hmt in range(HKT):
                    wd_h = til([P, IH, P], "w4", 4, dtype=dt.float16, name="wd_h")
                    nc.sync.dma_start(wd_h[:], Whi["Wd"][ihalf * (I // 2):(ihalf + 1) * (I // 2),
                                                         hmt * P:(hmt + 1) * P]
                                      .rearrange("(kt p) m -> p kt m", p=P))
                    wd_l = til([P, IH, P], "w4", 4, dtype=dt.float16, name="wd_l")
                    nc.sync.dma_start(wd_l[:], Wlo["Wd"][ihalf * (I // 2):(ihalf + 1) * (I // 2),
                                                         hmt * P:(hmt + 1) * P]
                                      .rearrange("(kt p) m -> p kt m", p=P))
                    psA = psum(name="psdA")
                    for kt in range(IH):
                        nc.tensor.matmul(psA[:, :TL], lhsT=wd_h[:, kt, :], rhs=sih[:, kt, :],
                                         start=(kt == 0), stop=(kt == IH - 1))
                    psB = psum(name="psdB")
                    for kt in range(IH):
                        nc.tensor.matmul(psB[:, :TL], lhsT=wd_l[:, kt, :], rhs=sih[:, kt, :],
                                         start=(kt == 0), stop=False)
                    for kt in range(IH):
                        nc.tensor.matmul(psB[:, :TL], lhsT=wd_h[:, kt, :], rhs=sil[:, kt, :],
                                         start=False, stop=(kt == IH - 1))
                    dsum = til([P, TL], "tdr", 6, name="dsum")
                    combine(psA, psB, dsum[:])
                    hfm = til([P, TL], "tdr", 6, name="hfm")
                    if ihalf == 0:
                        hres = til([P, TL], "tdr", 6, name="hres")
                        nc.sync.dma_start(hres[:], h_f_d[hmt * P:(hmt + 1) * P, :])
                        nc.vector.tensor_add(hfm[:], dsum[:], hres[:])
                        # stash partial in fp16 pair
                        split_fp16(hfm[:], hfh[:, hmt, :], hfl[:, hmt, :], (P, TL))
                    else:
                        prev = til([P, TL], "tdr", 6, name="prev")
                        nc.vector.scalar_tensor_tensor(prev[:], hfl[:, hmt, :], 1.0 / S,
                                                       hfh[:, hmt, :], op0=OP.mult, op1=OP.add)
                        nc.vector.tensor_add(hfm[:], dsum[:], prev[:])
                        split_fp16(hfm[:], hfh[:, hmt, :], hfl[:, hmt, :], (P, TL))
            # logits (batch-major) + argmax; reload Wout chunks per batch tile
            for bt in range(BT):
                lg = til([P, V], "lg", 2, name="lg")
                for nch in range(2):
                    wh = til([P, HKT, 512], "w8", 2, dtype=dt.float16, name=f"wouth{nch}")
                    nc.sync.dma_start(wh[:], Whi["Wout"][:, nch * 512:(nch + 1) * 512]
                                      .rearrange("(kt p) n -> p kt n", p=P))
                    wl = til([P, HKT, 512], "w8", 2, dtype=dt.float16, name=f"woutl{nch}")
                    nc.sync.dma_start(wl[:], Wlo["Wout"][:, nch * 512:(nch + 1) * 512]
                                      .rearrange("(kt p) n -> p kt n", p=P))
                    psA = psum(name="lgA")
                    for kt in range(HKT):
                        nc.tensor.matmul(psA[:], lhsT=hfh[:, kt, bt * P:(bt + 1) * P],
                                         rhs=wh[:, kt, :], start=(kt == 0), stop=(kt == HKT - 1))
                    psB = psum(name="lgB")
                    for kt in range(HKT):
                        nc.tensor.matmul(psB[:], lhsT=hfl[:, kt, bt * P:(bt + 1) * P],
                                         rhs=wh[:, kt, :], start=(kt == 0), stop=False)
                    for kt in range(HKT):
                        nc.tensor.matmul(psB[:], lhsT=hfh[:, kt, bt * P:(bt + 1) * P],
                                         rhs=wl[:, kt, :], start=False, stop=(kt == HKT - 1))
                    t = til([P, 512], "tdr", 6, name="lgt")
                    nc.scalar.mul(t[:], psB[:], 1.0 / S)
                    nc.vector.tensor_add(t[:], t[:], psA[:])
                    nc.vector.tensor_add(lg[:, nch * 512:(nch + 1) * 512], t[:],
                                         boutb[:, nch * 512:(nch + 1) * 512])
                mx = til([P, 8], "mx", 2, name="mx")
                nc.vector.max(mx[:], lg[:])
                idx = til([P, 8], "idx", 2, dtype=dt.uint32, name="idx")
                nc.vector.max_index(idx[:], mx[:], lg[:])
                tok = til([P, 1], "tok", 2 * BT + 1, dtype=dt.uint32, name="tok")
                nc.vector.tensor_copy(tok[:], idx[:, 0:1])
                tok_tiles[bt] = tok
                nc.sync.dma_start(toks_d[bt * P:(bt + 1) * P, p:p + 1],
                                  tok[:].bitcast(dt.int32))

        # ============ Phase B: step 0 ============
        x1h = til([P, HKT, TL], "act8", 4, dtype=dt.float16, name="x1h")
        x1l = til([P, HKT, TL], "act8", 4, dtype=dt.float16, name="x1l")
        rms_split_from_dram(x0_t_d, n1_sb, x1h, x1l)
        # k0 / v0: fp16x2 matmuls; drain transposed straight to the DRAM cache
        v0h = til([P, HKT, TL], "act8", 4, dtype=dt.float16, name="v0h")
        v0l = til([P, HKT, TL], "act8", 4, dtype=dt.float16, name="v0l")
        for w_name, is_v in (("Wk", False), ("Wv", True)):
            dst = Vc_d if is_v else Kc_d

            def drain_kv(mt, psA, psB, dst=dst, is_v=is_v):
                row = til([P, TL], "tdr", 6, name="kvrow")
                combine(psA, psB, row[:])
                if is_v:
                    split_fp16(row[:], v0h[:, mt, :], v0l[:, mt, :], (P, TL))
                tp = psum(name="kvtps")
                for bt in range(BT):
                    nc.tensor.transpose(tp[:, bt * P:(bt + 1) * P],
                                        row[:, bt * P:(bt + 1) * P], ident[:])
                row2 = til([P, TL], "tdr", 6, name="kvrow2")
                nc.vector.tensor_copy(row2[:], tp[:, :TL])
                for bt in range(BT):
                    nc.sync.dma_start(dst[bt * P:(bt + 1) * P, 0, mt * P:(mt + 1) * P],
                                      row2[:, bt * P:(bt + 1) * P])
            mm16(Whi[w_name], Wlo[w_name], x1h, x1l, drain_kv)

        # h = x0 + v0 @ Wo  (attention at step 0 is identity: ctx = v0)
        def drain_h_step0(mt, psA, psB):
            hs = til([P, TL], "tdr", 6, name="hs0")
            combine(psA, psB, hs[:])
            x0kt = til([P, TL], "tdr", 6, name="x0kt")
            nc.sync.dma_start(x0kt[:], x0_t_d[mt * P:(mt + 1) * P, :])
            nc.vector.tensor_add(hs[:], hs[:], x0kt[:])
            nc.sync.dma_start(h_f_d[mt * P:(mt + 1) * P, :], hs[:])
        mm16(Whi["Wo"], Wlo["Wo"], v0h, v0l, drain_h_step0)
        tok_tiles = [None] * BT
        mlp_and_logits(0, tok_tiles)

        # ============ Phase C: steps 1..7 ============
        for p in range(1, NSTEPS):
            cxh = til([P, HKT, TL], "act8", 4, dtype=dt.float16, name="cxh")
            cxl = til([P, HKT, TL], "act8", 4, dtype=dt.float16, name="cxl")
            ctx_b = [None] * BT
            for bt in range(BT):
                tokap = tok_tiles[bt][:, :1]
                cb = til([P, H], "g4", 5, name="cb")
                gather_rows(E_d, tokap, cb)
                transpose_b2f_to_dram(cb, bt, cur_f_d)
                qp = til([P, H], "g4", 5, name="qp")
                gather_rows(qEp_d[p], tokap, qp)
                kp = til([P, H], "g4", 5, name="kp")
                gather_rows(kEp_d[p], tokap, kp)
                vp = til([P, H], "g4", 5, name="vp")
                gather_rows(vE_d, tokap, vp)
                nc.sync.dma_start(Kc_d[bt * P:(bt + 1) * P, p, :], kp[:])
                nc.sync.dma_start(Vc_d[bt * P:(bt + 1) * P, p, :], vp[:])
                # ---- attention ----
                s_sb = til([P, P], "s", 2, name="s_sb")
                for j in range(p + 1):
                    if j < p:
                        Kj = til([P, H], "kv", 2, name="Kj")
                        nc.sync.dma_start(Kj[:], Kc_d[bt * P:(bt + 1) * P, j, :])
                    else:
                        Kj = kp
                    tmp = til([P, H], "tmp", 2, name="atmp")
                    nc.vector.tensor_mul(tmp[:], Kj[:], qp[:])
                    nc.vector.tensor_reduce(
                        s_sb[:, j * NH:(j + 1) * NH],
                        tmp[:].rearrange("p (h d) -> p h d", d=HD),
                        axis=AX.X, op=OP.add)
                m16 = til([P, NH], "m16", 4, name="m16")
                nc.vector.tensor_reduce(
                    m16[:], s_sb[:, :(p + 1) * NH].rearrange("p (j h) -> p h j", h=NH),
                    axis=AX.X, op=OP.max)
                e_sb = til([P, P], "s", 2, name="e_sb")
                nc.vector.tensor_tensor(
                    e_sb[:, :(p + 1) * NH], s_sb[:, :(p + 1) * NH],
                    m16[:].unsqueeze(1).to_broadcast([P, p + 1, NH]),
                    op=OP.subtract)
                nc.scalar.activation(e_sb[:, :(p + 1) * NH], e_sb[:, :(p + 1) * NH], AF.Exp)
                z16 = til([P, NH], "m16", 4, name="z16")
                nc.vector.tensor_reduce(
                    z16[:], e_sb[:, :(p + 1) * NH].rearrange("p (j h) -> p h j", h=NH),
                    axis=AX.X, op=OP.add)
                rz = til([P, NH], "m16", 4, name="rz")
                nc.vector.reciprocal(rz[:], z16[:])
                nc.vector.tensor_tensor(
                    e_sb[:, :(p + 1) * NH], e_sb[:, :(p + 1) * NH],
                    rz[:].unsqueeze(1).to_broadcast([P, p + 1, NH]),
                    op=OP.mult)
                cx = til([P, H], "cx", BT, name="cx")
                for j in range(p + 1):
                    if j < p:
                        Vj = til([P, H], "kv", 2, name="Vj")
                        nc.sync.dma_start(Vj[:], Vc_d[bt * P:(bt + 1) * P, j, :])
                    else:
                        Vj = vp
                    aj = e_sb[:, j * NH:(j + 1) * NH].unsqueeze(2).to_broadcast([P, NH, HD])
                    if j == 0:
                        nc.vector.tensor_tensor(
                            cx[:].rearrange("p (h d) -> p h d", d=HD),
                            Vj[:].rearrange("p (h d) -> p h d", d=HD), aj, op=OP.mult)
                    else:
                        tmp = til([P, H], "tmp", 2, name="atmp2")
                        nc.vector.tensor_tensor(
                            tmp[:].rearrange("p (h d) -> p h d", d=HD),
                            Vj[:].rearrange("p (h d) -> p h d", d=HD), aj, op=OP.mult)
                        nc.vector.tensor_add(cx[:], cx[:], tmp[:])
                ctx_b[bt] = cx
            transpose_b2f_split(ctx_b, cxh, cxl)

            # h = cur + ctx @ Wo
            def drain_h(mt, psA, psB):
                hs = til([P, TL], "tdr", 6, name="hs")
                combine(psA, psB, hs[:])
                ckt = til([P, TL], "tdr", 6, name="ckt")
                nc.sync.dma_start(ckt[:], cur_f_d[mt * P:(mt + 1) * P, :])
                nc.vector.tensor_add(hs[:], hs[:], ckt[:])
                nc.sync.dma_start(h_f_d[mt * P:(mt + 1) * P, :], hs[:])
            mm16(Whi["Wo"], Wlo["Wo"], cxh, cxl, drain_h)
            mlp_and_logits(p, tok_tiles)

    return nc


_CACHED = {}


def _build(TL):
    if TL in _CACHED:
        return _CACHED[TL]
    import concourse.bass as bass
    import concourse.tile as tile
    from concourse import bacc, mybir
    nc = bacc.Bacc("TRN2", target_bir_lowering=False, debug=False, num_devices=N_CORES)
    with tile.TileContext(nc) as tc:
        build_kernel(nc, tc, bass, mybir, TL)
    nc.compile()
    _CACHED[TL] = nc
    return nc


def make_in_maps(inputs, n_cores=N_CORES, TL=None):
    """Shard/augment the full inputs into per-core in_maps."""
    x0 = np.ascontiguousarray(np.asarray(inputs["chunk_hidden_states"], dtype=np.float32)[0])  # [T, H]
    T = x0.shape[0]
    if TL is None:
        TL = T // n_cores
    cq, sq, ck, sk = _rope_tables()
    shared = {
        "E": np.asarray(inputs["E"], np.float32),
        "n1": np.asarray(inputs["n1"], np.float32), "n2": np.asarray(inputs["n2"], np.float32),
        "bout": np.asarray(inputs["bout"], np.float32),
        "rope_cos_q": cq, "rope_sin_q": sq, "rope_cos_k": ck, "rope_sin_k": sk,
    }
    for w in WEIGHTS:
        wf = np.asarray(inputs[w], np.float32)
        hi = wf.astype(np.float16)
        lo = ((wf - hi.astype(np.float32)) * np.float32(S)).astype(np.float16)
        shared[w + "_hi"] = hi
        shared[w + "_lo"] = lo
    in_maps = []
    for c in range(n_cores):
        m = dict(shared)
        m["x0_t"] = np.ascontiguousarray(x0[c * TL:(c + 1) * TL, :].T)
        in_maps.append(m)
    return in_maps, TL


def kernel(**inputs):
    from concourse.bass_utils import run_bass_kernel_spmd
    in_maps, TL = make_in_maps(inputs)
    nc = _build(TL)
    res = run_bass_kernel_spmd(nc, in_maps, core_ids=list(range(N_CORES)))
    toks = np.concatenate([r["toks"] for r in res.results], axis=0)  # [T, 8]
    return toks.astype(np.int32)
